# revision 6
# baseline (speedup 1.0000x reference)
"""BiMambaBlock Trainium2 kernel (8-core SPMD via Bass/Tile), single launch.

Sharding: core = (b, dir, s) with b in {0,1} batch, dir in {fwd, bwd},
s in {0,1} half of d_inner (2048 -> 1024 per core).

Per core: layernorm (folded into W_in) -> in_proj -> causal depthwise
conv + silu -> x_proj -> dt_proj/softplus -> selective scan
(tensor_tensor_scan over time per (state n, 128-channel block); dBx on
DVE, prod on GPSIMD) -> gate by silu(z) -> out_proj partial -> partial
gating matmuls (own p @ Wg/Wv half, plus x @ per-core W half) for the
final bi-directional combine, which the host finishes (partial sums,
sigmoid, convex blend -- elementwise only, no host matmuls).

Everything on-device operates in time-transposed layout [feature, L].
bwd direction runs on host-reversed time; host un-reverses partials.
"""

import os
import sys

sys.path.insert(0, "/opt/trn_rl_repo")

SIM_SAFE = bool(os.environ.get("KERNEL_SIM_SAFE"))
# Which engine runs the prod (h*C) multiplies: "all" -> GPSIMD, "half",
# or "none" -> DVE. GPSIMD frees DVE (the bottleneck) in the scan phase.
POOL_PROD = os.environ.get("KERNEL_POOL_PROD", "all")

import numpy as np
import ml_dtypes

import concourse.bass as bass
import concourse.mybir as mybir
import concourse.tile as tile
from concourse import bacc
from concourse.bass_utils import run_bass_kernel_spmd

FP32 = mybir.dt.float32
BF16 = mybir.dt.bfloat16
AF = mybir.ActivationFunctionType
OP = mybir.AluOpType
BF = ml_dtypes.bfloat16

B, L, Dm, Di, N, R, KC = 2, 1024, 1024, 2048, 16, 64, 4
DiS = Di // 2  # 1024 channels per core
EPS = 1e-5
NCORES = 8

NXP = Di // 128        # 16 xp channel tiles
NSH = DiS // 128       # 8 shard channel tiles
NMD = Dm // 128        # 8 model-dim tiles


def build_launch1():
    nc = bacc.Bacc("TRN2", target_bir_lowering=False, debug=False,
                   num_devices=NCORES)
    xT = nc.dram_tensor("xT", [Dm, L], FP32, kind="ExternalInput")
    w_in = nc.dram_tensor("w_in", [Dm, 3072], BF16, kind="ExternalInput")
    w_in_c = nc.dram_tensor("w_in_c", [1, 3072], BF16, kind="ExternalInput")
    b_in = nc.dram_tensor("b_in", [3072, 1], FP32, kind="ExternalInput")
    conv_w = nc.dram_tensor("conv_w", [Di, KC], FP32, kind="ExternalInput")
    conv_b = nc.dram_tensor("conv_b", [Di, 1], FP32, kind="ExternalInput")
    w_xp = nc.dram_tensor("w_xp", [Di, 96], BF16, kind="ExternalInput")
    w_dt = nc.dram_tensor("w_dt", [R, DiS], BF16, kind="ExternalInput")
    b_dt = nc.dram_tensor("b_dt", [DiS, 1], FP32, kind="ExternalInput")
    a_mat = nc.dram_tensor("a_mat", [DiS, N], FP32, kind="ExternalInput")
    d_vec = nc.dram_tensor("d_vec", [DiS, 1], FP32, kind="ExternalInput")
    w_out = nc.dram_tensor("w_out", [DiS, Dm], BF16, kind="ExternalInput")
    wg_h = nc.dram_tensor("wg_h", [Dm, Dm], BF16, kind="ExternalInput")
    wv_h = nc.dram_tensor("wv_h", [Dm, Dm], BF16, kind="ExternalInput")
    wx_h = nc.dram_tensor("wx_h", [Dm, Dm], BF16, kind="ExternalInput")
    eye = nc.dram_tensor("eye", [2 * N, 2 * N * 128], BF16, kind="ExternalInput")
    ident = nc.dram_tensor("ident", [128, 128], BF16, kind="ExternalInput")
    dbl_sc = nc.dram_tensor("dbl_sc", [2 * N, L], BF16, kind="Internal")
    p_out = nc.dram_tensor("p_out", [Dm, L], BF16, kind="ExternalOutput")
    pre_a = nc.dram_tensor("pre_a", [Dm, L], FP32, kind="ExternalOutput")
    pre_b = nc.dram_tensor("pre_b", [Dm, L], FP32, kind="ExternalOutput")
    pre_x = nc.dram_tensor("pre_x", [Dm, L], FP32, kind="ExternalOutput")

    with tile.TileContext(nc) as tc:
        with (
            tc.tile_pool(name="pers", bufs=1) as pers,
            tc.tile_pool(name="bias", bufs=2) as biasp,
        ):
            # --- persistent tiles (whole-kernel lifetime): 14 MB ---
            silu_z = [pers.tile([128, L], BF16, name=f"sz{i}", tag=f"sz{i}")
                      for i in range(NSH)]
            xp_bf = [pers.tile([128, L], BF16, name=f"xp{i}", tag=f"xp{i}")
                     for i in range(NSH)]
            deltaT = [pers.tile([128, L], BF16, name=f"dl{i}", tag=f"dl{i}")
                      for i in range(NSH)]
            w_t = [pers.tile([128, L], BF16, name=f"wt{i}", tag=f"wt{i}")
                   for i in range(NSH)]
            y_acc = [pers.tile([128, L], BF16, name=f"ya{i}", tag=f"ya{i}")
                     for i in range(NSH)]
            xbf = [pers.tile([128, L], BF16, name=f"xb{i}", tag=f"xb{i}")
                   for i in range(NMD)]
            a_sb = [pers.tile([128, N], FP32, name=f"a{i}", tag=f"a{i}")
                    for i in range(NSH)]
            d_sb = [pers.tile([128, 1], FP32, name=f"d{i}", tag=f"d{i}")
                    for i in range(NSH)]
            ident_sb = pers.tile([128, 128], BF16, name="identsb",
                                 tag="identsb")
            nc.sync.dma_start(ident_sb[:], ident.ap())
            ones_f = pers.tile([1, 128], FP32, name="onesf", tag="onesf")
            ones_r = pers.tile([128, 1], FP32, name="onesr", tag="onesr")
            nc.vector.memset(ones_f[:], 1.0)
            nc.vector.memset(ones_r[:], 1.0)
            for i in range(NSH):
                nc.sync.dma_start(a_sb[i][:], a_mat.ap()[i * 128:(i + 1) * 128, :])
                nc.sync.dma_start(d_sb[i][:], d_vec.ap()[i * 128:(i + 1) * 128, :])

            # xp_hi: channels of the other Di half (x_proj input only)
            with tc.tile_pool(name="xph", bufs=1) as xph:
              xp_hi = [xph.tile([128, L], BF16, name=f"xh{i}", tag=f"xh{i}")
                       for i in range(NXP - NSH)]
              dbl_bf = xph.tile([96, L], BF16, name="dbl", tag="dbl")

              # ============ phase IP: LN stats + z1 + in_proj ============
              with (
                tc.tile_pool(name="ip", bufs=1) as ip,
                tc.tile_pool(name="wks", bufs=3) as wks,
                tc.tile_pool(name="cvt", bufs=2) as cvt,
              ):
                z1 = [ip.tile([128, L], BF16, name=f"z1{i}", tag=f"z1{i}")
                      for i in range(NMD)]
                mu = ip.tile([1, L], FP32, name="mu", tag="mu")
                rstd = ip.tile([1, L], FP32, name="rstd", tag="rstd")
                mr_row = ip.tile([1, L], BF16, name="mr", tag="mr")
                rstd_b = ip.tile([128, L], FP32, name="rstdb", tag="rstdb")
                wc = ip.tile([1, 3072], BF16, name="wc", tag="wc")
                nc.sync.dma_start(wc[:], w_in_c.ap())

                # ---- pass 1: stats (xti stay resident) ----
                with (
                    tc.tile_pool(name="sta", bufs=2) as sta,
                    tc.tile_pool(name="psst", bufs=1, space="PSUM") as psst,
                ):
                    ps_mu = psst.tile([1, L], FP32, name="psmu", tag="psmu")
                    ps_sq = psst.tile([1, L], FP32, name="pssq", tag="pssq")
                    for i in range(NMD):
                        xti = sta.tile([128, L], FP32, name="xti", tag="xti")
                        nc.sync.dma_start(xti[:],
                                          xT.ap()[i * 128:(i + 1) * 128, :])
                        x2i = sta.tile([128, L], FP32, name="x2i", tag="x2i",
                                       bufs=1)
                        nc.scalar.activation(x2i[:], xti[:], AF.Square)
                        for h in range(2):
                            sl = slice(h * 512, (h + 1) * 512)
                            nc.tensor.matmul(ps_mu[:, sl], ones_r[:],
                                             xti[:, sl],
                                             start=(i == 0), stop=(i == NMD - 1))
                            nc.tensor.matmul(ps_sq[:, sl], ones_r[:], x2i[:, sl],
                                             start=(i == 0), stop=(i == NMD - 1))
                    nc.scalar.mul(mu[:], ps_mu[:], 1.0 / Dm)
                    msq = sta.tile([1, L], FP32, name="strow", tag="strow", bufs=3)
                    nc.scalar.mul(msq[:], ps_sq[:], 1.0 / Dm)
                    mu2 = sta.tile([1, L], FP32, name="strow", tag="strow", bufs=3)
                    nc.vector.tensor_tensor(mu2[:], mu[:], mu[:], OP.mult)
                    var = sta.tile([1, L], FP32, name="strow", tag="strow", bufs=3)
                    nc.vector.tensor_tensor(var[:], msq[:], mu2[:], OP.subtract)
                    sd = sta.tile([1, L], FP32, name="strow", tag="strow", bufs=3)
                    eps_t = sta.tile([1, 1], FP32, name="epst", tag="epst")
                    nc.vector.memset(eps_t[:], EPS)
                    nc.scalar.activation(sd[:], var[:], AF.Sqrt, bias=eps_t[:])
                    nc.vector.reciprocal(rstd[:], sd[:])
                    nc.vector.tensor_tensor(mr_row[:], mu[:], rstd[:], OP.mult)
                    for h in range(2):
                        psb = psst.tile([128, 512], FP32, name="psb0", tag="psb0")
                        nc.tensor.matmul(psb[:], ones_f[:],
                                         rstd[:, h * 512:(h + 1) * 512],
                                         start=True, stop=True)
                        nc.scalar.copy(rstd_b[:, h * 512:(h + 1) * 512], psb[:])
                    # ---- pass 2: z1 = xT * rstd; xbf = bf16(xT) ----
                    for i in range(NMD):
                        xti = sta.tile([128, L], FP32, name="xti", tag="xti")
                        nc.sync.dma_start(xti[:],
                                          xT.ap()[i * 128:(i + 1) * 128, :])
                        nc.vector.tensor_tensor(z1[i][:], xti[:], rstd_b[:],
                                                OP.mult)
                        nc.scalar.copy(xbf[i][:], xti[:])

                # ---- in_proj: groups of 3 output tiles, stream weights ----
                with tc.tile_pool(name="psin", bufs=6, space="PSUM") as psin:
                    for mg in range(8):           # 8 groups x 3 mtiles
                        pst = [psin.tile([128, 512], FP32, name="psi",
                                         tag="psi") for _ in range(6)]
                        for kt in range(NMD):
                            wkt = wks.tile([128, 384], BF16, name="wkt",
                                           tag="wkt")
                            nc.sync.dma_start(
                                wkt[:],
                                w_in.ap()[kt * 128:(kt + 1) * 128,
                                          mg * 384:(mg + 1) * 384])
                            for m4 in range(3):
                                for h in range(2):
                                    nc.tensor.matmul(
                                        pst[m4 * 2 + h][:],
                                        wkt[:, m4 * 128:(m4 + 1) * 128],
                                        z1[kt][:, h * 512:(h + 1) * 512],
                                        start=(kt == 0), stop=False)
                        for m4 in range(3):
                            mt = mg * 3 + m4
                            for h in range(2):
                                nc.tensor.matmul(
                                    pst[m4 * 2 + h][:],
                                    wc[:, mt * 128:(mt + 1) * 128],
                                    mr_row[:, h * 512:(h + 1) * 512],
                                    start=False, stop=True)
                        for m4 in range(3):
                            mt = mg * 3 + m4
                            bi = biasp.tile([128, 1], FP32, name="bin",
                                            tag="bin")
                            nc.sync.dma_start(
                                bi[:], b_in.ap()[mt * 128:(mt + 1) * 128, :])
                            if mt < NXP:
                                # causal conv: 4 psum-accumulated diag matmuls
                                xpad = cvt.tile([128, L + 3], BF16,
                                                name="xpad", tag="xpad")
                                nc.vector.memset(xpad[:, 0:3], 0.0)
                                for h in range(2):
                                    nc.scalar.activation(
                                        xpad[:, 3 + h * 512:3 + (h + 1) * 512],
                                        pst[m4 * 2 + h][:], AF.Identity,
                                        bias=bi[:])
                                cw = biasp.tile([128, KC], FP32, name="cw",
                                                tag="cw")
                                cb = biasp.tile([128, 1], FP32, name="cb",
                                                tag="cb")
                                nc.sync.dma_start(
                                    cw[:], conv_w.ap()[mt * 128:(mt + 1) * 128, :])
                                nc.sync.dma_start(
                                    cb[:], conv_b.ap()[mt * 128:(mt + 1) * 128, :])
                                diags = []
                                for j in range(KC):
                                    dg = cvt.tile([128, 128], BF16,
                                                  name="diag", tag="diag",
                                                  bufs=8)
                                    nc.vector.tensor_scalar_mul(
                                        dg[:], ident_sb[:], cw[:, j:j + 1])
                                    diags.append(dg)
                                dst = (xp_bf[mt] if mt < NSH
                                       else xp_hi[mt - NSH])
                                for h in range(2):
                                    pcv = psin.tile([128, 512], FP32,
                                                    name="pcv", tag="pcv",
                                                    bufs=2)
                                    for j in range(KC):
                                        nc.tensor.matmul(
                                            pcv[:], diags[j][:],
                                            xpad[:, j + h * 512:
                                                 j + h * 512 + 512],
                                            start=(j == 0), stop=(j == KC - 1))
                                    cs = slice(h * 512, (h + 1) * 512)
                                    if SIM_SAFE:
                                        sg = cvt.tile([128, 512], BF16,
                                                      name="sg", tag="sg")
                                        nc.scalar.activation(
                                            sg[:], pcv[:], AF.Sigmoid,
                                            bias=cb[:])
                                        zz = cvt.tile([128, 512], BF16,
                                                      name="zz", tag="zz")
                                        nc.scalar.activation(
                                            zz[:], pcv[:], AF.Identity,
                                            bias=cb[:])
                                        nc.vector.tensor_tensor(
                                            dst[:, cs], zz[:], sg[:],
                                            OP.mult)
                                    else:
                                        nc.scalar.activation(
                                            dst[:, cs], pcv[:],
                                            AF.Silu, bias=cb[:])
                            else:
                                zt = silu_z[mt - NXP]
                                for h in range(2):
                                    cs = slice(h * 512, (h + 1) * 512)
                                    if SIM_SAFE:
                                        sg = cvt.tile([128, 512], BF16,
                                                      name="sg2", tag="sg2")
                                        nc.scalar.activation(
                                            sg[:], pst[m4 * 2 + h][:],
                                            AF.Sigmoid, bias=bi[:])
                                        zz = cvt.tile([128, 512], BF16,
                                                      name="zz2", tag="zz2")
                                        nc.scalar.activation(
                                            zz[:], pst[m4 * 2 + h][:],
                                            AF.Identity, bias=bi[:])
                                        nc.vector.tensor_tensor(
                                            zt[:, cs], zz[:], sg[:], OP.mult)
                                    else:
                                        nc.scalar.activation(
                                            zt[:, cs],
                                            pst[m4 * 2 + h][:], AF.Silu,
                                            bias=bi[:])

              # ============ phase XD: x_proj, dt_proj, w, y-init ============
              with (
                tc.tile_pool(name="xd", bufs=1) as xd,
                tc.tile_pool(name="psxd", bufs=2, space="PSUM") as psxd,
                tc.tile_pool(name="dbf", bufs=2) as dbfp,
              ):
                wx = [xd.tile([128, 96], BF16, name=f"wx{i}", tag=f"wx{i}")
                      for i in range(NXP)]
                for i in range(NXP):
                    nc.sync.dma_start(wx[i][:], w_xp.ap()[i * 128:(i + 1) * 128, :])
                for h in range(2):
                    psd = psxd.tile([96, 512], FP32, name="psd", tag="psd")
                    for kt in range(NXP):
                        src = xp_bf[kt] if kt < NSH else xp_hi[kt - NSH]
                        nc.tensor.matmul(psd[:], wx[kt][:],
                                         src[:, h * 512:(h + 1) * 512],
                                         start=(kt == 0), stop=(kt == NXP - 1))
                    nc.scalar.copy(dbl_bf[:, h * 512:(h + 1) * 512], psd[:])
                # bounce B/C rows through DRAM (partition-0 reload below)
                nc.sync.dma_start(dbl_sc.ap(), dbl_bf[R:R + 2 * N, :])

                wdt = xd.tile([R, DiS], BF16, name="wdt", tag="wdt")
                nc.sync.dma_start(wdt[:], w_dt.ap())
                for mt in range(NSH):
                    bdt = biasp.tile([128, 1], FP32, name="bdt", tag="bdt")
                    nc.sync.dma_start(bdt[:], b_dt.ap()[mt * 128:(mt + 1) * 128, :])
                    for h in range(2):
                        psdt = psxd.tile([128, 512], FP32, name="psdt",
                                         tag="psdt")
                        nc.tensor.matmul(psdt[:],
                                         wdt[:, mt * 128:(mt + 1) * 128],
                                         dbl_bf[0:R, h * 512:(h + 1) * 512],
                                         start=True, stop=True)
                        # softplus(x) = ln(1 + exp(x)); x << 80, no overflow
                        edt = dbfp.tile([128, 512], FP32, name="edt",
                                        tag="edt")
                        nc.scalar.activation(edt[:], psdt[:], AF.Exp,
                                             bias=bdt[:])
                        nc.scalar.activation(
                            deltaT[mt][:, h * 512:(h + 1) * 512],
                            edt[:], AF.Ln, bias=ones_r[:, 0:1])
                for mt in range(NSH):
                    nc.vector.tensor_tensor(w_t[mt][:], deltaT[mt][:],
                                            xp_bf[mt][:], OP.mult)
                    nc.vector.tensor_scalar_mul(y_acc[mt][:], xp_bf[mt][:],
                                                d_sb[mt][:])

            # ======= B/C broadcast hoist + scan, then FN =======
            with tc.tile_pool(name="bcp", bufs=1) as bcp:
                brep = [bcp.tile([128, L], BF16, name=f"br{n}", tag=f"br{n}")
                        for n in range(N)]
                crep = [bcp.tile([128, L], BF16, name=f"cr{n}", tag=f"cr{n}")
                        for n in range(N)]
                wxh = [bcp.tile([128, Dm], BF16, name=f"wxh{i}",
                                tag=f"wxh{i}")
                       for i in range(NMD)]
                for i in range(NMD):
                    nc.sync.dma_start(wxh[i][:],
                                      wx_h.ap()[i * 128:(i + 1) * 128, :])
                with (
                    tc.tile_pool(name="hoi", bufs=1) as hoistp,
                    tc.tile_pool(name="psr", bufs=2, space="PSUM") as psr,
                ):
                    bc_pack = hoistp.tile([2 * N, L], BF16, name="bcp",
                                          tag="bcp")
                    eye_sb = hoistp.tile([2 * N, 2 * N * 128], BF16,
                                         name="eyesb", tag="eyesb")
                    nc.sync.dma_start(eye_sb[:], eye.ap())
                    nc.sync.dma_start(bc_pack[:], dbl_sc.ap())
                    for j in range(2 * N):
                        rep = brep[j] if j < N else crep[j - N]
                        for h in range(2):
                            psb = psr.tile([128, 512], FP32, name="psrep",
                                           tag="psrep")
                            nc.tensor.matmul(
                                psb[:],
                                eye_sb[:, j * 128:(j + 1) * 128],
                                bc_pack[:, h * 512:(h + 1) * 512],
                                start=True, stop=True)
                            nc.scalar.copy(rep[:, h * 512:(h + 1) * 512],
                                           psb[:])

                # ============ phase SC: selective scan over n ============
                with (
                    tc.tile_pool(name="scan", bufs=3) as scanp,
                    tc.tile_pool(name="psy", bufs=1, space="PSUM") as psy,
                    tc.tile_pool(name="psx", bufs=2, space="PSUM") as psx,
                    tc.tile_pool(name="stg", bufs=3) as stgp,
                ):
                    first_group = True
                    for mts in ((0, 1, 2), (3, 4, 5), (6, 7)):
                        yps = {mt: psy.tile([128, L], FP32, name=f"psy{j}",
                                            tag=f"psy{j}")
                               for j, mt in enumerate(mts)}
                        for mt in mts:
                            for h in range(2):
                                nc.tensor.matmul(
                                    yps[mt][:, h * 512:(h + 1) * 512],
                                    ident_sb[:],
                                    y_acc[mt][:, h * 512:(h + 1) * 512],
                                    start=True, stop=False)
                        for n in range(N):
                            for mt in mts:
                                dA = scanp.tile([128, L], BF16, name="dA",
                                                tag="dA")
                                nc.scalar.activation(
                                    dA[:], deltaT[mt][:], AF.Exp,
                                    scale=a_sb[mt][:, n:n + 1])
                                dBx = scanp.tile([128, L], BF16, name="dBx",
                                                 tag="dBx")
                                nc.vector.tensor_tensor(dBx[:], w_t[mt][:],
                                                        brep[n][:], OP.mult)
                                hh = scanp.tile([128, L], BF16, name="hh",
                                                tag="hh")
                                nc.vector.tensor_tensor_scan(
                                    hh[:], dA[:], dBx[:], 0.0, OP.mult, OP.add)
                                prod = scanp.tile([128, L], BF16, name="prod",
                                                  tag="prod")
                                use_pool = (POOL_PROD == "all"
                                            or (POOL_PROD == "half"
                                                and n % 2 == 0))
                                peng = nc.gpsimd if use_pool else nc.vector
                                peng.tensor_tensor(prod[:], hh[:], crep[n][:],
                                                   OP.mult)
                                for h in range(2):
                                    nc.tensor.matmul(
                                        yps[mt][:, h * 512:(h + 1) * 512],
                                        ident_sb[:],
                                        prod[:, h * 512:(h + 1) * 512],
                                        start=False, stop=(n == N - 1))
                        for mt in mts:
                            nc.scalar.copy(y_acc[mt][:], yps[mt][:])
                            # gate: yf = y * silu(z), into the dead xp tile
                            nc.vector.tensor_tensor(xp_bf[mt][:], y_acc[mt][:],
                                                    silu_z[mt][:], OP.mult)
                        if first_group:
                            first_group = False
                            # x-term gating partial: x @ W-half (PE slack)
                            for mo2 in range(NMD):
                                for h in range(2):
                                    psq = psx.tile([128, 512], FP32,
                                                   name="psq", tag="psq")
                                    for mo in range(NMD):
                                        nc.tensor.matmul(
                                            psq[:],
                                            wxh[mo][:,
                                                    mo2 * 128:(mo2 + 1) * 128],
                                            xbf[mo][:, h * 512:(h + 1) * 512],
                                            start=(mo == 0),
                                            stop=(mo == NMD - 1))
                                    sq = stgp.tile([128, 512], FP32,
                                                   name="sq", tag="sq")
                                    nc.scalar.copy(sq[:], psq[:])
                                    nc.sync.dma_start(
                                        pre_x.ap()[mo2 * 128:(mo2 + 1) * 128,
                                                   h * 512:(h + 1) * 512],
                                        sq[:])

            # ============ phase FN: out_proj + partial gating ============
            with (
                tc.tile_pool(name="fin", bufs=1) as finp,
                tc.tile_pool(name="pso", bufs=4, space="PSUM") as pso_p,
                tc.tile_pool(name="stg2", bufs=3) as stg2,
            ):
                wo_sb = [finp.tile([128, Dm], BF16, name=f"wo{i}",
                                   tag=f"wo{i}")
                         for i in range(NSH)]
                for i in range(NSH):
                    nc.sync.dma_start(wo_sb[i][:],
                                      w_out.ap()[i * 128:(i + 1) * 128, :])
                wgv = [finp.tile([128, Dm], BF16, name=f"wgv{i}",
                                 tag=f"wgv{i}")
                       for i in range(2 * NMD)]
                for i in range(NMD):
                    nc.sync.dma_start(wgv[i][:],
                                      wg_h.ap()[i * 128:(i + 1) * 128, :])
                    nc.sync.dma_start(wgv[NMD + i][:],
                                      wv_h.ap()[i * 128:(i + 1) * 128, :])
                pbf = [finp.tile([128, L], BF16, name=f"pb{i}", tag=f"pb{i}")
                       for i in range(NMD)]
                for mo in range(NMD):
                    for h in range(2):
                        pso = pso_p.tile([128, 512], FP32, name="pso",
                                         tag="pso")
                        for kt in range(NSH):
                            nc.tensor.matmul(
                                pso[:], wo_sb[kt][:, mo * 128:(mo + 1) * 128],
                                xp_bf[kt][:, h * 512:(h + 1) * 512],
                                start=(kt == 0), stop=(kt == NSH - 1))
                        nc.scalar.copy(pbf[mo][:, h * 512:(h + 1) * 512],
                                       pso[:])
                    nc.sync.dma_start(p_out.ap()[mo * 128:(mo + 1) * 128, :],
                                      pbf[mo][:])
                # partial gating: own p @ Wg-half and @ Wv-half
                for gv in range(2):
                    dst = pre_a if gv == 0 else pre_b
                    for mo2 in range(NMD):
                        for h in range(2):
                            psg = pso_p.tile([128, 512], FP32, name="psg",
                                             tag="psg")
                            for mo in range(NMD):
                                nc.tensor.matmul(
                                    psg[:],
                                    wgv[gv * NMD + mo][
                                        :, mo2 * 128:(mo2 + 1) * 128],
                                    pbf[mo][:, h * 512:(h + 1) * 512],
                                    start=(mo == 0), stop=(mo == NMD - 1))
                            sg2 = stg2.tile([128, 512], FP32,
                                            name="sg2", tag="sg2")
                            nc.scalar.copy(sg2[:], psg[:])
                            nc.sync.dma_start(
                                dst.ap()[mo2 * 128:(mo2 + 1) * 128,
                                         h * 512:(h + 1) * 512],
                                sg2[:])

    nc.compile()
    return nc


# ------------------------------------------------------------------- host
_cache = {}


def _get_nc(which=1):
    if which not in _cache:
        _cache[which] = build_launch1()
    return _cache[which]


def prep_launch1_inmaps(x, ln_w, ln_b, W_in, b_in, conv_w, conv_b, W_xproj,
                        W_dt, b_dt, A_log, D, W_out, b_out, Wg, Wv):
    """Build the 8 per-core input dicts."""
    in_maps = []
    eye_np = np.zeros((2 * N, 2 * N * 128), np.float32)
    for j in range(2 * N):
        eye_np[j, j * 128:(j + 1) * 128] = 1.0
    eye_np = eye_np.astype(BF)
    ident_np = np.eye(128, dtype=np.float32).astype(BF)
    xf = [np.ascontiguousarray(x[b].T) for b in range(B)]           # [Dm, L]
    xr = [np.ascontiguousarray(x[b, ::-1].T) for b in range(B)]     # reversed
    wg_bf = [Wg[dr * Dm:(dr + 1) * Dm, :].astype(BF) for dr in range(2)]
    wv_bf = [Wv[dr * Dm:(dr + 1) * Dm, :].astype(BF) for dr in range(2)]
    for core in range(NCORES):
        b, dr, s = core >> 2, (core >> 1) & 1, core & 1
        sl = slice(s * DiS, (s + 1) * DiS)
        W_eff = ln_w[dr][:, None] * W_in[dr]                         # [Dm, 4096]
        b_eff = ln_b[dr] @ W_in[dr] + b_in[dr]                       # [4096]
        # xp channel permutation: this core's Di-shard channels come first
        perm = np.concatenate([np.arange(s * DiS, (s + 1) * DiS),
                               np.arange((1 - s) * DiS, (2 - s) * DiS)])
        cols = np.concatenate([perm, Di + s * DiS + np.arange(DiS)])
        Wc = W_eff[:, cols]                                          # [Dm, 3072]
        in_maps.append({
            "xT": (xf if dr == 0 else xr)[b],
            "w_in": Wc.astype(BF),
            "w_in_c": (-Wc.sum(0, keepdims=True)).astype(BF),
            "b_in": b_eff[cols][:, None].astype(np.float32),
            "conv_w": conv_w[dr][perm].astype(np.float32),
            "conv_b": conv_b[dr][perm][:, None].astype(np.float32),
            "w_xp": W_xproj[dr][perm].astype(BF),
            "w_dt": W_dt[dr][:, sl].astype(BF),
            "b_dt": b_dt[dr][sl][:, None].astype(np.float32),
            "a_mat": (-np.exp(A_log[dr][sl])).astype(np.float32),
            "d_vec": D[dr][sl][:, None].astype(np.float32),
            "w_out": W_out[dr][sl, :].astype(BF),
            "wg_h": wg_bf[dr],
            "wv_h": wv_bf[dr],
            "wx_h": wg_bf[dr] if s == 0 else wv_bf[dr],
            "eye": eye_np,
            "ident": ident_np,
        })
    return in_maps, xf


def postprocess(res1, x, b_out, Wg, bg, Wv, bv):
    """Host combine: sums of partials, sigmoid gate, convex blend."""
    idx = lambda b, dr, s: (b << 2) | (dr << 1) | s
    out = np.empty((B, L, Dm), np.float32)
    f32 = np.float32
    bias_g = (b_out[0] @ Wg[:Dm] + b_out[1] @ Wg[Dm:] + bg).astype(f32)
    bias_v = (b_out[0] @ Wv[:Dm] + b_out[1] @ Wv[Dm:] + bv).astype(f32)
    for b in range(B):
        pf = (res1[idx(b, 0, 0)]["p_out"].astype(f32)
              + res1[idx(b, 0, 1)]["p_out"].astype(f32))
        pb = (res1[idx(b, 1, 0)]["p_out"].astype(f32)
              + res1[idx(b, 1, 1)]["p_out"].astype(f32))[:, ::-1]
        xb = x[b].T.astype(f32)                               # [Dm, L]
        fwd = xb + b_out[0][:, None] + pf
        bwd = xb + b_out[1][:, None] + pb
        s_sum = fwd + bwd
        pre_g = (res1[idx(b, 0, 0)]["pre_a"] + res1[idx(b, 0, 1)]["pre_a"]
                 + res1[idx(b, 1, 0)]["pre_a"][:, ::-1]
                 + res1[idx(b, 1, 1)]["pre_a"][:, ::-1]
                 + res1[idx(b, 0, 0)]["pre_x"]
                 + res1[idx(b, 1, 0)]["pre_x"][:, ::-1])
        pre_v = (res1[idx(b, 0, 0)]["pre_b"] + res1[idx(b, 0, 1)]["pre_b"]
                 + res1[idx(b, 1, 0)]["pre_b"][:, ::-1]
                 + res1[idx(b, 1, 1)]["pre_b"][:, ::-1]
                 + res1[idx(b, 0, 1)]["pre_x"]
                 + res1[idx(b, 1, 1)]["pre_x"][:, ::-1])
        g = 1.0 / (1.0 + np.exp(-(pre_g + bias_g[:, None])))
        v = pre_v + bias_v[:, None]
        out[b] = (0.5 * (g * (v - s_sum) + s_sum)).T
    return out


def kernel(x, ln_w, ln_b, W_in, b_in, conv_w, conv_b, W_xproj, W_dt, b_dt,
           A_log, D, W_out, b_out, Wg, bg, Wv, bv):
    x = np.asarray(x, np.float32)
    args = [np.asarray(a, np.float32) for a in
            (ln_w, ln_b, W_in, b_in, conv_w, conv_b, W_xproj, W_dt, b_dt,
             A_log, D, W_out, b_out)]
    Wg, bg, Wv, bv = (np.asarray(a, np.float32) for a in (Wg, bg, Wv, bv))

    in1, xf = prep_launch1_inmaps(x, *args, Wg, Wv)
    nc1 = _get_nc(1)
    res1 = run_bass_kernel_spmd(nc1, in1, core_ids=list(range(NCORES))).results
    return postprocess(res1, x, args[-1], Wg, bg, Wv, bv)


# revision 14
# speedup vs baseline: 1.1563x; 1.1563x over previous
"""BiMambaBlock Trainium2 kernel (8-core SPMD via Bass/Tile), single launch.

Sharding: core = (b, dir, s) with b in {0,1} batch, dir in {fwd, bwd},
s in {0,1} half of d_inner (2048 -> 1024 per core).

Per core: layernorm (folded into W_in) -> in_proj -> causal depthwise
conv + silu -> x_proj -> dt_proj/softplus -> selective scan
(tensor_tensor_scan over time per (state n, 128-channel block); dBx on
DVE, prod on GPSIMD) -> gate by silu(z) -> out_proj partial -> partial
gating matmuls (own p @ Wg/Wv half, plus x @ per-core W half) for the
final bi-directional combine, which the host finishes (partial sums,
sigmoid, convex blend -- elementwise only, no host matmuls).

Everything on-device operates in time-transposed layout [feature, L].
bwd direction runs on host-reversed time; host un-reverses partials.
"""

import os
import sys

sys.path.insert(0, "/opt/trn_rl_repo")

SIM_SAFE = bool(os.environ.get("KERNEL_SIM_SAFE"))
# Fraction (in eighths) of the prod (h*C) multiplies run on GPSIMD to
# free DVE (the scan-phase bottleneck). GPSIMD under SBUF contention runs
# ~4-5us per [128,1024] op, so it can only absorb ~60%.
POOL_EIGHTHS = int(os.environ.get("KERNEL_POOL_EIGHTHS", "3"))
_POOL_PAT = [i * POOL_EIGHTHS % 8 < POOL_EIGHTHS for i in range(8)]
_POOL_PAT = ([True] * POOL_EIGHTHS + [False] * (8 - POOL_EIGHTHS))

import numpy as np
import ml_dtypes

import concourse.bass as bass
import concourse.mybir as mybir
import concourse.tile as tile
from concourse import bacc
from concourse.bass_utils import run_bass_kernel_spmd

FP32 = mybir.dt.float32
BF16 = mybir.dt.bfloat16
AF = mybir.ActivationFunctionType
OP = mybir.AluOpType
BF = ml_dtypes.bfloat16

B, L, Dm, Di, N, R, KC = 2, 1024, 1024, 2048, 16, 64, 4
DiS = Di // 2  # 1024 channels per core
EPS = 1e-5
NCORES = 8

NXP = Di // 128        # 16 xp channel tiles
NSH = DiS // 128       # 8 shard channel tiles
NMD = Dm // 128        # 8 model-dim tiles


def build_launch1():
    nc = bacc.Bacc("TRN2", target_bir_lowering=False, debug=False,
                   num_devices=NCORES)
    xT = nc.dram_tensor("xT", [Dm, L], FP32, kind="ExternalInput")
    w_in = nc.dram_tensor("w_in", [Dm, 3072], BF16, kind="ExternalInput")
    w_in_c = nc.dram_tensor("w_in_c", [1, 3072], BF16, kind="ExternalInput")
    b_in = nc.dram_tensor("b_in", [3072, 1], FP32, kind="ExternalInput")
    conv_w = nc.dram_tensor("conv_w", [Di, KC], FP32, kind="ExternalInput")
    conv_b = nc.dram_tensor("conv_b", [Di, 1], FP32, kind="ExternalInput")
    w_xp = nc.dram_tensor("w_xp", [Di, 96], BF16, kind="ExternalInput")
    w_dt = nc.dram_tensor("w_dt", [R, DiS], BF16, kind="ExternalInput")
    b_dt = nc.dram_tensor("b_dt", [DiS, 1], FP32, kind="ExternalInput")
    a_mat = nc.dram_tensor("a_mat", [DiS, N], FP32, kind="ExternalInput")
    d_vec = nc.dram_tensor("d_vec", [DiS, 1], FP32, kind="ExternalInput")
    w_out = nc.dram_tensor("w_out", [DiS, Dm], BF16, kind="ExternalInput")
    wg_h = nc.dram_tensor("wg_h", [Dm, Dm], BF16, kind="ExternalInput")
    wv_h = nc.dram_tensor("wv_h", [Dm, Dm], BF16, kind="ExternalInput")
    wx_h = nc.dram_tensor("wx_h", [Dm, Dm], BF16, kind="ExternalInput")
    ident = nc.dram_tensor("ident", [128, 128], BF16, kind="ExternalInput")
    p_out = nc.dram_tensor("p_out", [Dm, L], BF16, kind="ExternalOutput")
    pre_a = nc.dram_tensor("pre_a", [Dm, L], FP32, kind="ExternalOutput")
    pre_b = nc.dram_tensor("pre_b", [Dm, L], FP32, kind="ExternalOutput")
    pre_x = nc.dram_tensor("pre_x", [Dm, L], FP32, kind="ExternalOutput")

    with tile.TileContext(nc) as tc:
        with (
            tc.tile_pool(name="pers", bufs=1) as pers,
            tc.tile_pool(name="bias", bufs=2) as biasp,
            tc.tile_pool(name="dsc", bufs=1, space="DRAM") as dscp,
        ):
            dbl_dram = dscp.tile([2 * N, L], BF16, name="dbldr", tag="dbldr")
            # --- persistent tiles (whole-kernel lifetime): 14 MB ---
            silu_z = [pers.tile([128, L], BF16, name=f"sz{i}", tag=f"sz{i}")
                      for i in range(NSH)]
            xp_bf = [pers.tile([128, L], BF16, name=f"xp{i}", tag=f"xp{i}")
                     for i in range(NSH)]
            deltaT = [pers.tile([128, L], BF16, name=f"dl{i}", tag=f"dl{i}")
                      for i in range(NSH)]
            w_t = [pers.tile([128, L], BF16, name=f"wt{i}", tag=f"wt{i}")
                   for i in range(NSH)]
            y_acc = [pers.tile([128, L], BF16, name=f"ya{i}", tag=f"ya{i}")
                     for i in range(NSH)]
            xbf = [pers.tile([128, L], BF16, name=f"xb{i}", tag=f"xb{i}")
                   for i in range(NMD)]
            a_sb = [pers.tile([128, N], FP32, name=f"a{i}", tag=f"a{i}")
                    for i in range(NSH)]
            d_sb = [pers.tile([128, 1], FP32, name=f"d{i}", tag=f"d{i}")
                    for i in range(NSH)]
            ident_sb = pers.tile([128, 128], BF16, name="identsb",
                                 tag="identsb")
            nc.sync.dma_start(ident_sb[:], ident.ap())
            ones_f = pers.tile([1, 128], FP32, name="onesf", tag="onesf")
            ones_r = pers.tile([128, 1], FP32, name="onesr", tag="onesr")
            nc.vector.memset(ones_f[:], 1.0)
            nc.vector.memset(ones_r[:], 1.0)
            for i in range(NSH):
                nc.sync.dma_start(a_sb[i][:], a_mat.ap()[i * 128:(i + 1) * 128, :])
                nc.sync.dma_start(d_sb[i][:], d_vec.ap()[i * 128:(i + 1) * 128, :])

            # xp_hi: channels of the other Di half (x_proj input only)
            with tc.tile_pool(name="xph", bufs=1) as xph:
              xp_hi = [xph.tile([128, L], BF16, name=f"xh{i}", tag=f"xh{i}")
                       for i in range(NXP - NSH)]
              dbl_bf = xph.tile([96, L], BF16, name="dbl", tag="dbl")
              wx = [xph.tile([128, 96], BF16, name=f"wx{i}", tag=f"wx{i}")
                    for i in range(NXP)]
              for i in range(NXP):
                  nc.sync.dma_start(wx[i][:], w_xp.ap()[i * 128:(i + 1) * 128, :])
              wdt = xph.tile([R, DiS], BF16, name="wdt", tag="wdt")
              nc.sync.dma_start(wdt[:], w_dt.ap())

              # ============ phase IP: LN stats + z1 + in_proj ============
              with (
                tc.tile_pool(name="ip", bufs=1) as ip,
                tc.tile_pool(name="wks", bufs=3) as wks,
                tc.tile_pool(name="cvt", bufs=2) as cvt,
              ):
                z1 = [ip.tile([128, L], BF16, name=f"z1{i}", tag=f"z1{i}")
                      for i in range(NMD)]
                mu = ip.tile([1, L], FP32, name="mu", tag="mu")
                rstd = ip.tile([1, L], FP32, name="rstd", tag="rstd")
                mr_row = ip.tile([1, L], BF16, name="mr", tag="mr")
                rstd_b = ip.tile([128, L], FP32, name="rstdb", tag="rstdb")
                wc = ip.tile([1, 3072], BF16, name="wc", tag="wc")
                nc.sync.dma_start(wc[:], w_in_c.ap())

                # ---- pass 1: stats ----
                with (
                    tc.tile_pool(name="sta", bufs=2) as sta,
                    tc.tile_pool(name="psst", bufs=1, space="PSUM") as psst,
                ):
                    ps_mu = psst.tile([1, L], FP32, name="psmu", tag="psmu")
                    ps_sq = psst.tile([1, L], FP32, name="pssq", tag="pssq")
                    for i in range(NMD):
                        xti = sta.tile([128, L], FP32, name="xti", tag="xti",
                                       bufs=3)
                        nc.sync.dma_start(xti[:],
                                          xT.ap()[i * 128:(i + 1) * 128, :])
                        x2i = sta.tile([128, L], FP32, name="x2i", tag="x2i",
                                       bufs=1)
                        nc.scalar.activation(x2i[:], xti[:], AF.Square)
                        for h in range(2):
                            sl = slice(h * 512, (h + 1) * 512)
                            nc.tensor.matmul(ps_mu[:, sl], ones_r[:],
                                             xti[:, sl],
                                             start=(i == 0), stop=(i == NMD - 1))
                            nc.tensor.matmul(ps_sq[:, sl], ones_r[:], x2i[:, sl],
                                             start=(i == 0), stop=(i == NMD - 1))
                    nc.scalar.mul(mu[:], ps_mu[:], 1.0 / Dm)
                    msq = sta.tile([1, L], FP32, name="strow", tag="strow", bufs=3)
                    nc.scalar.mul(msq[:], ps_sq[:], 1.0 / Dm)
                    mu2 = sta.tile([1, L], FP32, name="strow", tag="strow", bufs=3)
                    nc.vector.tensor_tensor(mu2[:], mu[:], mu[:], OP.mult)
                    var = sta.tile([1, L], FP32, name="strow", tag="strow", bufs=3)
                    nc.vector.tensor_tensor(var[:], msq[:], mu2[:], OP.subtract)
                    eps_t = sta.tile([1, 1], FP32, name="epst", tag="epst")
                    nc.vector.memset(eps_t[:], EPS)
                    lnv = sta.tile([1, L], FP32, name="strow", tag="strow",
                                   bufs=3)
                    nc.scalar.activation(lnv[:], var[:], AF.Ln, bias=eps_t[:])
                    nc.scalar.activation(rstd[:], lnv[:], AF.Exp, scale=-0.5)
                    nc.vector.tensor_tensor(mr_row[:], mu[:], rstd[:], OP.mult)
                    for h in range(2):
                        psb = psst.tile([128, 512], FP32, name="psb0", tag="psb0")
                        nc.tensor.matmul(psb[:], ones_f[:],
                                         rstd[:, h * 512:(h + 1) * 512],
                                         start=True, stop=True)
                        nc.scalar.copy(rstd_b[:, h * 512:(h + 1) * 512], psb[:])
                    # ---- pass 2: z1 = xT * rstd; xbf = bf16(xT) ----
                    for i in range(NMD):
                        xti = sta.tile([128, L], FP32, name="xti", tag="xti",
                                       bufs=3)
                        nc.sync.dma_start(xti[:],
                                          xT.ap()[i * 128:(i + 1) * 128, :])
                        nc.vector.tensor_tensor(z1[i][:], xti[:], rstd_b[:],
                                                OP.mult)
                        nc.scalar.copy(xbf[i][:], xti[:])

                # ---- in_proj: groups of 2 output tiles, stream weights ----
                # xp tiles (mt 0..15) first, then xproj/dt interlude, then z.
                def in_proj_group(mts2, psin):
                    pst = [psin.tile([128, 512], FP32, name="psi",
                                     tag="psi", bufs=4) for _ in range(4)]
                    for kt in range(NMD):
                        wkt = wks.tile([128, 256], BF16, name="wkt",
                                       tag="wkt")
                        nc.sync.dma_start(
                            wkt[:],
                            w_in.ap()[kt * 128:(kt + 1) * 128,
                                      mts2[0] * 128:(mts2[0] + 2) * 128])
                        for m2 in range(2):
                            for h in range(2):
                                nc.tensor.matmul(
                                    pst[m2 * 2 + h][:],
                                    wkt[:, m2 * 128:(m2 + 1) * 128],
                                    z1[kt][:, h * 512:(h + 1) * 512],
                                    start=(kt == 0), stop=False)
                    for m2, mt in enumerate(mts2):
                        for h in range(2):
                            nc.tensor.matmul(
                                pst[m2 * 2 + h][:],
                                wc[:, mt * 128:(mt + 1) * 128],
                                mr_row[:, h * 512:(h + 1) * 512],
                                start=False, stop=True)
                    for m2, mt in enumerate(mts2):
                        bi = biasp.tile([128, 1], FP32, name="bin",
                                        tag="bin")
                        nc.sync.dma_start(
                            bi[:], b_in.ap()[mt * 128:(mt + 1) * 128, :])
                        if mt < NXP:
                            # causal conv: 4 psum-accumulated diag matmuls
                            xpad = cvt.tile([128, L + 3], BF16,
                                            name="xpad", tag="xpad")
                            nc.vector.memset(xpad[:, 0:3], 0.0)
                            for h in range(2):
                                nc.scalar.activation(
                                    xpad[:, 3 + h * 512:3 + (h + 1) * 512],
                                    pst[m2 * 2 + h][:], AF.Identity,
                                    bias=bi[:])
                            cw = biasp.tile([128, KC], FP32, name="cw",
                                            tag="cw")
                            cb = biasp.tile([128, 1], FP32, name="cb",
                                            tag="cb")
                            nc.sync.dma_start(
                                cw[:], conv_w.ap()[mt * 128:(mt + 1) * 128, :])
                            nc.sync.dma_start(
                                cb[:], conv_b.ap()[mt * 128:(mt + 1) * 128, :])
                            diags = []
                            for j in range(KC):
                                dg = cvt.tile([128, 128], BF16,
                                              name="diag", tag="diag",
                                              bufs=8)
                                nc.vector.tensor_scalar_mul(
                                    dg[:], ident_sb[:], cw[:, j:j + 1])
                                diags.append(dg)
                            dst = (xp_bf[mt] if mt < NSH
                                   else xp_hi[mt - NSH])
                            for h in range(2):
                                pcv = psin.tile([128, 512], FP32,
                                                name="pcv", tag="pcv",
                                                bufs=2)
                                for j in range(KC):
                                    nc.tensor.matmul(
                                        pcv[:], diags[j][:],
                                        xpad[:, j + h * 512:
                                             j + h * 512 + 512],
                                        start=(j == 0), stop=(j == KC - 1))
                                cs = slice(h * 512, (h + 1) * 512)
                                if SIM_SAFE:
                                    sg = cvt.tile([128, 512], BF16,
                                                  name="sg", tag="sg")
                                    nc.scalar.activation(
                                        sg[:], pcv[:], AF.Sigmoid,
                                        bias=cb[:])
                                    zz = cvt.tile([128, 512], BF16,
                                                  name="zz", tag="zz")
                                    nc.scalar.activation(
                                        zz[:], pcv[:], AF.Identity,
                                        bias=cb[:])
                                    nc.vector.tensor_tensor(
                                        dst[:, cs], zz[:], sg[:],
                                        OP.mult)
                                else:
                                    nc.scalar.activation(
                                        dst[:, cs], pcv[:],
                                        AF.Silu, bias=cb[:])
                        else:
                            zt = silu_z[mt - NXP]
                            for h in range(2):
                                cs = slice(h * 512, (h + 1) * 512)
                                if SIM_SAFE:
                                    sg = cvt.tile([128, 512], BF16,
                                                  name="sg2", tag="sg2")
                                    nc.scalar.activation(
                                        sg[:], pst[m2 * 2 + h][:],
                                        AF.Sigmoid, bias=bi[:])
                                    zz = cvt.tile([128, 512], BF16,
                                                  name="zz2", tag="zz2")
                                    nc.scalar.activation(
                                        zz[:], pst[m2 * 2 + h][:],
                                        AF.Identity, bias=bi[:])
                                    nc.vector.tensor_tensor(
                                        zt[:, cs], zz[:], sg[:], OP.mult)
                                else:
                                    nc.scalar.activation(
                                        zt[:, cs],
                                        pst[m2 * 2 + h][:], AF.Silu,
                                        bias=bi[:])

                with tc.tile_pool(name="psin", bufs=4, space="PSUM") as psin:
                    for g in range(8):            # xp tiles mt 0..15
                        in_proj_group((2 * g, 2 * g + 1), psin)

                    # ---- xproj (needs all xp) ----
                    for h in range(2):
                        psda = psin.tile([128, 512], FP32, name="aux",
                                         tag="aux", bufs=2)
                        psd = psda[0:96, :]
                        for kt in range(NXP):
                            srct = xp_bf[kt] if kt < NSH else xp_hi[kt - NSH]
                            nc.tensor.matmul(psd[:], wx[kt][:],
                                             srct[:, h * 512:(h + 1) * 512],
                                             start=(kt == 0),
                                             stop=(kt == NXP - 1))
                        nc.scalar.copy(dbl_bf[:, h * 512:(h + 1) * 512],
                                       psd[:])
                    # bounce B/C rows through DRAM (broadcast reload below)
                    nc.sync.dma_start(dbl_dram[:], dbl_bf[R:R + 2 * N, :])

                    # ---- dt_proj + softplus + w/y-init, per mt ----
                    for mt in range(NSH):
                        bdt = biasp.tile([128, 1], FP32, name="bdt",
                                         tag="bdt")
                        nc.sync.dma_start(
                            bdt[:], b_dt.ap()[mt * 128:(mt + 1) * 128, :])
                        for h in range(2):
                            psdt = psin.tile([128, 512], FP32, name="aux",
                                             tag="aux", bufs=2)
                            nc.tensor.matmul(
                                psdt[:], wdt[:, mt * 128:(mt + 1) * 128],
                                dbl_bf[0:R, h * 512:(h + 1) * 512],
                                start=True, stop=True)
                            # softplus(x) = ln(1 + exp(x)); x << 80
                            edt = cvt.tile([128, 512], FP32, name="edt",
                                           tag="edt", bufs=4)
                            nc.scalar.activation(edt[:], psdt[:], AF.Exp,
                                                 bias=bdt[:])
                            nc.scalar.activation(
                                deltaT[mt][:, h * 512:(h + 1) * 512],
                                edt[:], AF.Ln, bias=ones_r[:, 0:1])
                        nc.vector.tensor_tensor(w_t[mt][:], deltaT[mt][:],
                                                xp_bf[mt][:], OP.mult)
                        nc.vector.tensor_scalar_mul(y_acc[mt][:],
                                                    xp_bf[mt][:],
                                                    d_sb[mt][:])

                    for g in range(4):            # z tiles mt 16..23
                        in_proj_group((16 + 2 * g, 17 + 2 * g), psin)

            # ======= B/C broadcast hoist + scan, then FN =======
            with tc.tile_pool(name="bcp", bufs=1) as bcp:
                brep = [bcp.tile([128, L], BF16, name=f"br{n}", tag=f"br{n}")
                        for n in range(N)]
                crep = [bcp.tile([128, L], BF16, name=f"cr{n}", tag=f"cr{n}")
                        for n in range(N)]
                wxh = [bcp.tile([128, Dm], BF16, name=f"wxh{i}",
                                tag=f"wxh{i}")
                       for i in range(NMD)]
                for i in range(NMD):
                    nc.sync.dma_start(wxh[i][:],
                                      wx_h.ap()[i * 128:(i + 1) * 128, :])
                # fill brep/crep via partition-broadcast DMA from DRAM
                for j in range(2 * N):
                    rep = brep[j] if j < N else crep[j - N]
                    nc.sync.dma_start(
                        rep[:],
                        dbl_dram[j:j + 1, :].broadcast_to([128, L]))

                # ============ phase SC: selective scan over n ============
                with (
                    tc.tile_pool(name="scan", bufs=2) as scanp,
                    tc.tile_pool(name="psy", bufs=1, space="PSUM") as psy,
                    tc.tile_pool(name="psx", bufs=2, space="PSUM") as psx,
                    tc.tile_pool(name="stg", bufs=3) as stgp,
                ):
                    first_group = True
                    uidx = 0
                    for mts in ((0, 1, 2), (3, 4, 5), (6, 7)):
                        yps = {mt: psy.tile([128, L], FP32, name=f"psy{j}",
                                            tag=f"psy{j}")
                               for j, mt in enumerate(mts)}
                        for mt in mts:
                            for h in range(2):
                                nc.tensor.matmul(
                                    yps[mt][:, h * 512:(h + 1) * 512],
                                    ident_sb[:],
                                    y_acc[mt][:, h * 512:(h + 1) * 512],
                                    start=True, stop=False)
                        for n in range(N):
                            for mt in mts:
                                dA = scanp.tile([128, L], BF16, name="dA",
                                                tag="dA")
                                nc.scalar.activation(
                                    dA[:], deltaT[mt][:], AF.Exp,
                                    scale=a_sb[mt][:, n:n + 1])
                                dBx = scanp.tile([128, L], BF16, name="dBx",
                                                 tag="dBx")
                                nc.vector.tensor_tensor(dBx[:], w_t[mt][:],
                                                        brep[n][:], OP.mult)
                                hh = scanp.tile([128, L], BF16, name="hh",
                                                tag="hh", bufs=4)
                                nc.vector.tensor_tensor_scan(
                                    hh[:], dA[:], dBx[:], 0.0, OP.mult, OP.add)
                                prod = scanp.tile([128, L], BF16, name="prod",
                                                  tag="prod", bufs=4)
                                use_pool = _POOL_PAT[uidx % 8]
                                uidx += 1
                                peng = nc.gpsimd if use_pool else nc.vector
                                peng.tensor_tensor(prod[:], hh[:], crep[n][:],
                                                   OP.mult)
                                for h in range(2):
                                    nc.tensor.matmul(
                                        yps[mt][:, h * 512:(h + 1) * 512],
                                        ident_sb[:],
                                        prod[:, h * 512:(h + 1) * 512],
                                        start=False, stop=(n == N - 1))
                        for mt in mts:
                            nc.scalar.copy(y_acc[mt][:], yps[mt][:])
                            # gate: yf = y * silu(z), into the dead xp tile
                            nc.vector.tensor_tensor(xp_bf[mt][:], y_acc[mt][:],
                                                    silu_z[mt][:], OP.mult)
                        if first_group:
                            first_group = False
                            # x-term gating partial: x @ W-half (PE slack)
                            for mo2 in range(NMD):
                                for h in range(2):
                                    psq = psx.tile([128, 512], FP32,
                                                   name="psq", tag="psq")
                                    for mo in range(NMD):
                                        nc.tensor.matmul(
                                            psq[:],
                                            wxh[mo][:,
                                                    mo2 * 128:(mo2 + 1) * 128],
                                            xbf[mo][:, h * 512:(h + 1) * 512],
                                            start=(mo == 0),
                                            stop=(mo == NMD - 1))
                                    sq = stgp.tile([128, 512], FP32,
                                                   name="sq", tag="sq")
                                    nc.scalar.copy(sq[:], psq[:])
                                    nc.sync.dma_start(
                                        pre_x.ap()[mo2 * 128:(mo2 + 1) * 128,
                                                   h * 512:(h + 1) * 512],
                                        sq[:])

            # ============ phase FN: out_proj + partial gating ============
            with (
                tc.tile_pool(name="fin", bufs=1) as finp,
                tc.tile_pool(name="pso", bufs=4, space="PSUM") as pso_p,
                tc.tile_pool(name="stg2", bufs=3) as stg2,
            ):
                wo_sb = [finp.tile([128, Dm], BF16, name=f"wo{i}",
                                   tag=f"wo{i}")
                         for i in range(NSH)]
                for i in range(NSH):
                    nc.sync.dma_start(wo_sb[i][:],
                                      w_out.ap()[i * 128:(i + 1) * 128, :])
                wgv = [finp.tile([128, Dm], BF16, name=f"wgv{i}",
                                 tag=f"wgv{i}")
                       for i in range(2 * NMD)]
                for i in range(NMD):
                    nc.sync.dma_start(wgv[i][:],
                                      wg_h.ap()[i * 128:(i + 1) * 128, :])
                    nc.sync.dma_start(wgv[NMD + i][:],
                                      wv_h.ap()[i * 128:(i + 1) * 128, :])
                pbf = [finp.tile([128, L], BF16, name=f"pb{i}", tag=f"pb{i}")
                       for i in range(NMD)]
                for mo in range(NMD):
                    for h in range(2):
                        pso = pso_p.tile([128, 512], FP32, name="pso",
                                         tag="pso")
                        for kt in range(NSH):
                            nc.tensor.matmul(
                                pso[:], wo_sb[kt][:, mo * 128:(mo + 1) * 128],
                                xp_bf[kt][:, h * 512:(h + 1) * 512],
                                start=(kt == 0), stop=(kt == NSH - 1))
                        nc.scalar.copy(pbf[mo][:, h * 512:(h + 1) * 512],
                                       pso[:])
                    nc.sync.dma_start(p_out.ap()[mo * 128:(mo + 1) * 128, :],
                                      pbf[mo][:])
                # partial gating: own p @ Wg-half and @ Wv-half
                for gv in range(2):
                    dst = pre_a if gv == 0 else pre_b
                    for mo2 in range(NMD):
                        for h in range(2):
                            psg = pso_p.tile([128, 512], FP32, name="psg",
                                             tag="psg")
                            for mo in range(NMD):
                                nc.tensor.matmul(
                                    psg[:],
                                    wgv[gv * NMD + mo][
                                        :, mo2 * 128:(mo2 + 1) * 128],
                                    pbf[mo][:, h * 512:(h + 1) * 512],
                                    start=(mo == 0), stop=(mo == NMD - 1))
                            sg2 = stg2.tile([128, 512], FP32,
                                            name="sg2", tag="sg2")
                            nc.scalar.copy(sg2[:], psg[:])
                            nc.sync.dma_start(
                                dst.ap()[mo2 * 128:(mo2 + 1) * 128,
                                         h * 512:(h + 1) * 512],
                                sg2[:])

    nc.compile()
    return nc


# ------------------------------------------------------------------- host
_cache = {}


def _get_nc(which=1):
    if which not in _cache:
        _cache[which] = build_launch1()
    return _cache[which]


def prep_launch1_inmaps(x, ln_w, ln_b, W_in, b_in, conv_w, conv_b, W_xproj,
                        W_dt, b_dt, A_log, D, W_out, b_out, Wg, Wv):
    """Build the 8 per-core input dicts."""
    in_maps = []
    ident_np = np.eye(128, dtype=np.float32).astype(BF)
    xf = [np.ascontiguousarray(x[b].T) for b in range(B)]           # [Dm, L]
    xr = [np.ascontiguousarray(x[b, ::-1].T) for b in range(B)]     # reversed
    wg_bf = [Wg[dr * Dm:(dr + 1) * Dm, :].astype(BF) for dr in range(2)]
    wv_bf = [Wv[dr * Dm:(dr + 1) * Dm, :].astype(BF) for dr in range(2)]
    for core in range(NCORES):
        b, dr, s = core >> 2, (core >> 1) & 1, core & 1
        sl = slice(s * DiS, (s + 1) * DiS)
        W_eff = ln_w[dr][:, None] * W_in[dr]                         # [Dm, 4096]
        b_eff = ln_b[dr] @ W_in[dr] + b_in[dr]                       # [4096]
        # xp channel permutation: this core's Di-shard channels come first
        perm = np.concatenate([np.arange(s * DiS, (s + 1) * DiS),
                               np.arange((1 - s) * DiS, (2 - s) * DiS)])
        cols = np.concatenate([perm, Di + s * DiS + np.arange(DiS)])
        Wc = W_eff[:, cols]                                          # [Dm, 3072]
        in_maps.append({
            "xT": (xf if dr == 0 else xr)[b],
            "w_in": Wc.astype(BF),
            "w_in_c": (-Wc.sum(0, keepdims=True)).astype(BF),
            "b_in": b_eff[cols][:, None].astype(np.float32),
            "conv_w": conv_w[dr][perm].astype(np.float32),
            "conv_b": conv_b[dr][perm][:, None].astype(np.float32),
            "w_xp": W_xproj[dr][perm].astype(BF),
            "w_dt": W_dt[dr][:, sl].astype(BF),
            "b_dt": b_dt[dr][sl][:, None].astype(np.float32),
            "a_mat": (-np.exp(A_log[dr][sl])).astype(np.float32),
            "d_vec": D[dr][sl][:, None].astype(np.float32),
            "w_out": W_out[dr][sl, :].astype(BF),
            "wg_h": wg_bf[dr],
            "wv_h": wv_bf[dr],
            "wx_h": wg_bf[dr] if s == 0 else wv_bf[dr],
            "ident": ident_np,
        })
    return in_maps, xf


def postprocess(res1, x, b_out, Wg, bg, Wv, bv):
    """Host combine: sums of partials, sigmoid gate, convex blend."""
    idx = lambda b, dr, s: (b << 2) | (dr << 1) | s
    out = np.empty((B, L, Dm), np.float32)
    f32 = np.float32
    bias_g = (b_out[0] @ Wg[:Dm] + b_out[1] @ Wg[Dm:] + bg).astype(f32)
    bias_v = (b_out[0] @ Wv[:Dm] + b_out[1] @ Wv[Dm:] + bv).astype(f32)
    for b in range(B):
        pf = (res1[idx(b, 0, 0)]["p_out"].astype(f32)
              + res1[idx(b, 0, 1)]["p_out"].astype(f32))
        pb = (res1[idx(b, 1, 0)]["p_out"].astype(f32)
              + res1[idx(b, 1, 1)]["p_out"].astype(f32))[:, ::-1]
        xb = x[b].T.astype(f32)                               # [Dm, L]
        fwd = xb + b_out[0][:, None] + pf
        bwd = xb + b_out[1][:, None] + pb
        s_sum = fwd + bwd
        pre_g = (res1[idx(b, 0, 0)]["pre_a"] + res1[idx(b, 0, 1)]["pre_a"]
                 + res1[idx(b, 1, 0)]["pre_a"][:, ::-1]
                 + res1[idx(b, 1, 1)]["pre_a"][:, ::-1]
                 + res1[idx(b, 0, 0)]["pre_x"]
                 + res1[idx(b, 1, 0)]["pre_x"][:, ::-1])
        pre_v = (res1[idx(b, 0, 0)]["pre_b"] + res1[idx(b, 0, 1)]["pre_b"]
                 + res1[idx(b, 1, 0)]["pre_b"][:, ::-1]
                 + res1[idx(b, 1, 1)]["pre_b"][:, ::-1]
                 + res1[idx(b, 0, 1)]["pre_x"]
                 + res1[idx(b, 1, 1)]["pre_x"][:, ::-1])
        g = 1.0 / (1.0 + np.exp(-(pre_g + bias_g[:, None])))
        v = pre_v + bias_v[:, None]
        out[b] = (0.5 * (g * (v - s_sum) + s_sum)).T
    return out


def kernel(x, ln_w, ln_b, W_in, b_in, conv_w, conv_b, W_xproj, W_dt, b_dt,
           A_log, D, W_out, b_out, Wg, bg, Wv, bv):
    x = np.asarray(x, np.float32)
    args = [np.asarray(a, np.float32) for a in
            (ln_w, ln_b, W_in, b_in, conv_w, conv_b, W_xproj, W_dt, b_dt,
             A_log, D, W_out, b_out)]
    Wg, bg, Wv, bv = (np.asarray(a, np.float32) for a in (Wg, bg, Wv, bv))

    in1, xf = prep_launch1_inmaps(x, *args, Wg, Wv)
    nc1 = _get_nc(1)
    res1 = run_bass_kernel_spmd(nc1, in1, core_ids=list(range(NCORES))).results
    return postprocess(res1, x, args[-1], Wg, bg, Wv, bv)


# revision 15
# speedup vs baseline: 1.2781x; 1.1053x over previous
"""BiMambaBlock Trainium2 kernel (8-core SPMD via Bass/Tile), single launch.

Sharding: core = (b, dir, s) with b in {0,1} batch, dir in {fwd, bwd},
s in {0,1} half of d_inner (2048 -> 1024 per core).

Per core: layernorm (folded into W_in) -> in_proj -> causal depthwise
conv + silu -> x_proj -> dt_proj/softplus -> selective scan
(tensor_tensor_scan over time per (state n, 128-channel block); dBx on
DVE, prod on GPSIMD) -> gate by silu(z) -> out_proj partial -> partial
gating matmuls (own p @ Wg/Wv half, plus x @ per-core W half) for the
final bi-directional combine, which the host finishes (partial sums,
sigmoid, convex blend -- elementwise only, no host matmuls).

Everything on-device operates in time-transposed layout [feature, L].
bwd direction runs on host-reversed time; host un-reverses partials.
"""

import os
import sys

sys.path.insert(0, "/opt/trn_rl_repo")

SIM_SAFE = bool(os.environ.get("KERNEL_SIM_SAFE"))
# Fraction (in eighths) of the prod (h*C) multiplies run on GPSIMD to
# free DVE (the scan-phase bottleneck). GPSIMD under SBUF contention runs
# ~4-5us per [128,1024] op, so it can only absorb ~60%.
POOL_EIGHTHS = int(os.environ.get("KERNEL_POOL_EIGHTHS", "0"))
_POOL_PAT = [i * POOL_EIGHTHS % 8 < POOL_EIGHTHS for i in range(8)]
_POOL_PAT = ([True] * POOL_EIGHTHS + [False] * (8 - POOL_EIGHTHS))

import numpy as np
import ml_dtypes

import concourse.bass as bass
import concourse.mybir as mybir
import concourse.tile as tile
from concourse import bacc
from concourse.bass_utils import run_bass_kernel_spmd

FP32 = mybir.dt.float32
BF16 = mybir.dt.bfloat16
AF = mybir.ActivationFunctionType
OP = mybir.AluOpType
BF = ml_dtypes.bfloat16

B, L, Dm, Di, N, R, KC = 2, 1024, 1024, 2048, 16, 64, 4
DiS = Di // 2  # 1024 channels per core
EPS = 1e-5
NCORES = 8

NXP = Di // 128        # 16 xp channel tiles
NSH = DiS // 128       # 8 shard channel tiles
NMD = Dm // 128        # 8 model-dim tiles


def build_launch1():
    nc = bacc.Bacc("TRN2", target_bir_lowering=False, debug=False,
                   num_devices=NCORES)
    xT = nc.dram_tensor("xT", [Dm, L], FP32, kind="ExternalInput")
    w_in = nc.dram_tensor("w_in", [Dm, 3072], BF16, kind="ExternalInput")
    w_in_c = nc.dram_tensor("w_in_c", [1, 3072], BF16, kind="ExternalInput")
    b_in = nc.dram_tensor("b_in", [3072, 1], FP32, kind="ExternalInput")
    conv_w = nc.dram_tensor("conv_w", [Di, KC], FP32, kind="ExternalInput")
    conv_b = nc.dram_tensor("conv_b", [Di, 1], FP32, kind="ExternalInput")
    w_xp = nc.dram_tensor("w_xp", [Di, 96], BF16, kind="ExternalInput")
    w_dt = nc.dram_tensor("w_dt", [R, DiS], BF16, kind="ExternalInput")
    b_dt = nc.dram_tensor("b_dt", [DiS, 1], FP32, kind="ExternalInput")
    a_mat = nc.dram_tensor("a_mat", [DiS, N], FP32, kind="ExternalInput")
    d_vec = nc.dram_tensor("d_vec", [DiS, 1], FP32, kind="ExternalInput")
    w_out = nc.dram_tensor("w_out", [DiS, Dm], BF16, kind="ExternalInput")
    wg_h = nc.dram_tensor("wg_h", [Dm, Dm], BF16, kind="ExternalInput")
    wv_h = nc.dram_tensor("wv_h", [Dm, Dm], BF16, kind="ExternalInput")
    wx_h = nc.dram_tensor("wx_h", [Dm, Dm], BF16, kind="ExternalInput")
    ident = nc.dram_tensor("ident", [128, 128], BF16, kind="ExternalInput")
    p_out = nc.dram_tensor("p_out", [Dm, L], BF16, kind="ExternalOutput")
    pre_a = nc.dram_tensor("pre_a", [Dm, L], FP32, kind="ExternalOutput")
    pre_b = nc.dram_tensor("pre_b", [Dm, L], FP32, kind="ExternalOutput")
    pre_x = nc.dram_tensor("pre_x", [Dm, L], FP32, kind="ExternalOutput")

    with tile.TileContext(nc) as tc:
        with (
            tc.tile_pool(name="pers", bufs=1) as pers,
            tc.tile_pool(name="bias", bufs=2) as biasp,
            tc.tile_pool(name="dsc", bufs=1, space="DRAM") as dscp,
        ):
            dbl_dram = dscp.tile([2 * N, L], BF16, name="dbldr", tag="dbldr")
            # --- persistent tiles (whole-kernel lifetime): 14 MB ---
            silu_z = [pers.tile([128, L], BF16, name=f"sz{i}", tag=f"sz{i}")
                      for i in range(NSH)]
            xp_bf = [pers.tile([128, L], BF16, name=f"xp{i}", tag=f"xp{i}")
                     for i in range(NSH)]
            deltaT = [pers.tile([128, L], BF16, name=f"dl{i}", tag=f"dl{i}")
                      for i in range(NSH)]
            w_t = [pers.tile([128, L], BF16, name=f"wt{i}", tag=f"wt{i}")
                   for i in range(NSH)]
            y_acc = [pers.tile([128, L], BF16, name=f"ya{i}", tag=f"ya{i}")
                     for i in range(NSH)]
            xbf = [pers.tile([128, L], BF16, name=f"xb{i}", tag=f"xb{i}")
                   for i in range(NMD)]
            a_sb = [pers.tile([128, N], FP32, name=f"a{i}", tag=f"a{i}")
                    for i in range(NSH)]
            d_sb = [pers.tile([128, 1], FP32, name=f"d{i}", tag=f"d{i}")
                    for i in range(NSH)]
            ident_sb = pers.tile([128, 128], BF16, name="identsb",
                                 tag="identsb")
            nc.sync.dma_start(ident_sb[:], ident.ap())
            ones_f = pers.tile([1, 128], FP32, name="onesf", tag="onesf")
            ones_r = pers.tile([128, 1], FP32, name="onesr", tag="onesr")
            nc.vector.memset(ones_f[:], 1.0)
            nc.vector.memset(ones_r[:], 1.0)
            for i in range(NSH):
                nc.sync.dma_start(a_sb[i][:], a_mat.ap()[i * 128:(i + 1) * 128, :])
                nc.sync.dma_start(d_sb[i][:], d_vec.ap()[i * 128:(i + 1) * 128, :])

            # xp_hi: channels of the other Di half (x_proj input only)
            with tc.tile_pool(name="xph", bufs=1) as xph:
              xp_hi = [xph.tile([128, L], BF16, name=f"xh{i}", tag=f"xh{i}")
                       for i in range(NXP - NSH)]
              dbl_bf = xph.tile([96, L], BF16, name="dbl", tag="dbl")
              wx = [xph.tile([128, 96], BF16, name=f"wx{i}", tag=f"wx{i}")
                    for i in range(NXP)]
              for i in range(NXP):
                  nc.sync.dma_start(wx[i][:], w_xp.ap()[i * 128:(i + 1) * 128, :])
              wdt = xph.tile([R, DiS], BF16, name="wdt", tag="wdt")
              nc.sync.dma_start(wdt[:], w_dt.ap())

              # ============ phase IP: LN stats + z1 + in_proj ============
              with (
                tc.tile_pool(name="ip", bufs=1) as ip,
                tc.tile_pool(name="wks", bufs=3) as wks,
                tc.tile_pool(name="cvt", bufs=2) as cvt,
              ):
                z1 = [ip.tile([128, L], BF16, name=f"z1{i}", tag=f"z1{i}")
                      for i in range(NMD)]
                mu = ip.tile([1, L], FP32, name="mu", tag="mu")
                rstd = ip.tile([1, L], FP32, name="rstd", tag="rstd")
                mr_row = ip.tile([1, L], BF16, name="mr", tag="mr")
                rstd_b = ip.tile([128, L], BF16, name="rstdb", tag="rstdb")
                wc = ip.tile([1, 3072], BF16, name="wc", tag="wc")
                nc.sync.dma_start(wc[:], w_in_c.ap())

                # ---- pass 1: stats ----
                with (
                    tc.tile_pool(name="sta", bufs=2) as sta,
                    tc.tile_pool(name="psst", bufs=1, space="PSUM") as psst,
                ):
                    ps_mu = psst.tile([1, L], FP32, name="psmu", tag="psmu")
                    ps_sq = psst.tile([1, L], FP32, name="pssq", tag="pssq")
                    for i in range(NMD):
                        xti = sta.tile([128, L], FP32, name="xti", tag="xti",
                                       bufs=3)
                        nc.sync.dma_start(xti[:],
                                          xT.ap()[i * 128:(i + 1) * 128, :])
                        x2i = sta.tile([128, L], FP32, name="x2i", tag="x2i",
                                       bufs=1)
                        nc.scalar.activation(x2i[:], xti[:], AF.Square)
                        nc.vector.tensor_copy(xbf[i][:], xti[:])
                        for h in range(2):
                            sl = slice(h * 512, (h + 1) * 512)
                            nc.tensor.matmul(ps_mu[:, sl], ones_r[:],
                                             xti[:, sl],
                                             start=(i == 0), stop=(i == NMD - 1))
                            nc.tensor.matmul(ps_sq[:, sl], ones_r[:], x2i[:, sl],
                                             start=(i == 0), stop=(i == NMD - 1))
                    nc.scalar.mul(mu[:], ps_mu[:], 1.0 / Dm)
                    msq = sta.tile([1, L], FP32, name="strow", tag="strow", bufs=3)
                    nc.scalar.mul(msq[:], ps_sq[:], 1.0 / Dm)
                    mu2 = sta.tile([1, L], FP32, name="strow", tag="strow", bufs=3)
                    nc.vector.tensor_tensor(mu2[:], mu[:], mu[:], OP.mult)
                    var = sta.tile([1, L], FP32, name="strow", tag="strow", bufs=3)
                    nc.vector.tensor_tensor(var[:], msq[:], mu2[:], OP.subtract)
                    eps_t = sta.tile([1, 1], FP32, name="epst", tag="epst")
                    nc.vector.memset(eps_t[:], EPS)
                    lnv = sta.tile([1, L], FP32, name="strow", tag="strow",
                                   bufs=3)
                    nc.scalar.activation(lnv[:], var[:], AF.Ln, bias=eps_t[:])
                    nc.scalar.activation(rstd[:], lnv[:], AF.Exp, scale=-0.5)
                    nc.vector.tensor_tensor(mr_row[:], mu[:], rstd[:], OP.mult)
                    for h in range(2):
                        psb = psst.tile([128, 512], FP32, name="psb0", tag="psb0")
                        nc.tensor.matmul(psb[:], ones_f[:],
                                         rstd[:, h * 512:(h + 1) * 512],
                                         start=True, stop=True)
                        nc.scalar.copy(rstd_b[:, h * 512:(h + 1) * 512], psb[:])
                    # ---- pass 2: z1 = bf16(xT) * rstd (no re-DMA) ----
                    for i in range(NMD):
                        nc.vector.tensor_tensor(z1[i][:], xbf[i][:],
                                                rstd_b[:], OP.mult)

                # ---- in_proj: groups of 2 output tiles, stream weights ----
                # xp tiles (mt 0..15) first, then xproj/dt interlude, then z.
                def in_proj_group(mts2, psin):
                    pst = [psin.tile([128, 512], FP32, name="psi",
                                     tag="psi", bufs=4) for _ in range(4)]
                    for kt in range(NMD):
                        wkt = wks.tile([128, 256], BF16, name="wkt",
                                       tag="wkt")
                        nc.sync.dma_start(
                            wkt[:],
                            w_in.ap()[kt * 128:(kt + 1) * 128,
                                      mts2[0] * 128:(mts2[0] + 2) * 128])
                        for m2 in range(2):
                            for h in range(2):
                                nc.tensor.matmul(
                                    pst[m2 * 2 + h][:],
                                    wkt[:, m2 * 128:(m2 + 1) * 128],
                                    z1[kt][:, h * 512:(h + 1) * 512],
                                    start=(kt == 0), stop=False)
                    for m2, mt in enumerate(mts2):
                        for h in range(2):
                            nc.tensor.matmul(
                                pst[m2 * 2 + h][:],
                                wc[:, mt * 128:(mt + 1) * 128],
                                mr_row[:, h * 512:(h + 1) * 512],
                                start=False, stop=True)
                    for m2, mt in enumerate(mts2):
                        bi = biasp.tile([128, 1], FP32, name="bin",
                                        tag="bin")
                        nc.sync.dma_start(
                            bi[:], b_in.ap()[mt * 128:(mt + 1) * 128, :])
                        if mt < NXP:
                            # causal conv: 4 psum-accumulated diag matmuls
                            xpad = cvt.tile([128, L + 3], BF16,
                                            name="xpad", tag="xpad")
                            nc.vector.memset(xpad[:, 0:3], 0.0)
                            for h in range(2):
                                nc.scalar.activation(
                                    xpad[:, 3 + h * 512:3 + (h + 1) * 512],
                                    pst[m2 * 2 + h][:], AF.Identity,
                                    bias=bi[:])
                            cw = biasp.tile([128, KC], FP32, name="cw",
                                            tag="cw")
                            cb = biasp.tile([128, 1], FP32, name="cb",
                                            tag="cb")
                            nc.sync.dma_start(
                                cw[:], conv_w.ap()[mt * 128:(mt + 1) * 128, :])
                            nc.sync.dma_start(
                                cb[:], conv_b.ap()[mt * 128:(mt + 1) * 128, :])
                            diags = []
                            for j in range(KC):
                                dg = cvt.tile([128, 128], BF16,
                                              name="diag", tag="diag",
                                              bufs=8)
                                nc.vector.tensor_scalar_mul(
                                    dg[:], ident_sb[:], cw[:, j:j + 1])
                                diags.append(dg)
                            dst = (xp_bf[mt] if mt < NSH
                                   else xp_hi[mt - NSH])
                            for h in range(2):
                                pcv = psin.tile([128, 512], FP32,
                                                name="pcv", tag="pcv",
                                                bufs=2)
                                for j in range(KC):
                                    nc.tensor.matmul(
                                        pcv[:], diags[j][:],
                                        xpad[:, j + h * 512:
                                             j + h * 512 + 512],
                                        start=(j == 0), stop=(j == KC - 1))
                                cs = slice(h * 512, (h + 1) * 512)
                                if SIM_SAFE:
                                    sg = cvt.tile([128, 512], BF16,
                                                  name="sg", tag="sg")
                                    nc.scalar.activation(
                                        sg[:], pcv[:], AF.Sigmoid,
                                        bias=cb[:])
                                    zz = cvt.tile([128, 512], BF16,
                                                  name="zz", tag="zz")
                                    nc.scalar.activation(
                                        zz[:], pcv[:], AF.Identity,
                                        bias=cb[:])
                                    nc.vector.tensor_tensor(
                                        dst[:, cs], zz[:], sg[:],
                                        OP.mult)
                                else:
                                    nc.scalar.activation(
                                        dst[:, cs], pcv[:],
                                        AF.Silu, bias=cb[:])
                        else:
                            zt = silu_z[mt - NXP]
                            for h in range(2):
                                cs = slice(h * 512, (h + 1) * 512)
                                if SIM_SAFE:
                                    sg = cvt.tile([128, 512], BF16,
                                                  name="sg2", tag="sg2")
                                    nc.scalar.activation(
                                        sg[:], pst[m2 * 2 + h][:],
                                        AF.Sigmoid, bias=bi[:])
                                    zz = cvt.tile([128, 512], BF16,
                                                  name="zz2", tag="zz2")
                                    nc.scalar.activation(
                                        zz[:], pst[m2 * 2 + h][:],
                                        AF.Identity, bias=bi[:])
                                    nc.vector.tensor_tensor(
                                        zt[:, cs], zz[:], sg[:], OP.mult)
                                else:
                                    nc.scalar.activation(
                                        zt[:, cs],
                                        pst[m2 * 2 + h][:], AF.Silu,
                                        bias=bi[:])

                with tc.tile_pool(name="psin", bufs=4, space="PSUM") as psin:
                    for g in range(8):            # xp tiles mt 0..15
                        in_proj_group((2 * g, 2 * g + 1), psin)

                    # ---- xproj (needs all xp) ----
                    for h in range(2):
                        psda = psin.tile([128, 512], FP32, name="aux",
                                         tag="aux", bufs=2)
                        psd = psda[0:96, :]
                        for kt in range(NXP):
                            srct = xp_bf[kt] if kt < NSH else xp_hi[kt - NSH]
                            nc.tensor.matmul(psd[:], wx[kt][:],
                                             srct[:, h * 512:(h + 1) * 512],
                                             start=(kt == 0),
                                             stop=(kt == NXP - 1))
                        nc.scalar.copy(dbl_bf[:, h * 512:(h + 1) * 512],
                                       psd[:])
                    # bounce B/C rows through DRAM (broadcast reload below)
                    nc.sync.dma_start(dbl_dram[:], dbl_bf[R:R + 2 * N, :])

                    # ---- dt_proj + softplus + w/y-init, interleaved
                    # with the z-half in_proj groups (PE/ACT overlap) ----
                    for blk in range(4):
                        for mt in (2 * blk, 2 * blk + 1):
                            bdt = biasp.tile([128, 1], FP32, name="bdt",
                                             tag="bdt")
                            nc.sync.dma_start(
                                bdt[:], b_dt.ap()[mt * 128:(mt + 1) * 128, :])
                            for h in range(2):
                                psdt = psin.tile([128, 512], FP32,
                                                 name="aux", tag="aux",
                                                 bufs=2)
                                nc.tensor.matmul(
                                    psdt[:], wdt[:, mt * 128:(mt + 1) * 128],
                                    dbl_bf[0:R, h * 512:(h + 1) * 512],
                                    start=True, stop=True)
                                # softplus(x) = ln(1 + exp(x)); x << 80
                                edt = cvt.tile([128, 512], FP32, name="edt",
                                               tag="edt", bufs=4)
                                nc.scalar.activation(edt[:], psdt[:], AF.Exp,
                                                     bias=bdt[:])
                                nc.scalar.activation(
                                    deltaT[mt][:, h * 512:(h + 1) * 512],
                                    edt[:], AF.Ln, bias=ones_r[:, 0:1])
                            nc.vector.tensor_tensor(w_t[mt][:],
                                                    deltaT[mt][:],
                                                    xp_bf[mt][:], OP.mult)
                            nc.vector.tensor_scalar_mul(y_acc[mt][:],
                                                        xp_bf[mt][:],
                                                        d_sb[mt][:])
                        in_proj_group((16 + 2 * blk, 17 + 2 * blk), psin)

            # ======= B/C broadcast hoist + scan, then FN =======
            with tc.tile_pool(name="bcp", bufs=1) as bcp:
                brep = [bcp.tile([128, L], BF16, name=f"br{n}", tag=f"br{n}")
                        for n in range(N)]
                crep = [bcp.tile([128, L], BF16, name=f"cr{n}", tag=f"cr{n}")
                        for n in range(N)]
                wxh = [bcp.tile([128, Dm], BF16, name=f"wxh{i}",
                                tag=f"wxh{i}")
                       for i in range(NMD)]
                for i in range(NMD):
                    nc.sync.dma_start(wxh[i][:],
                                      wx_h.ap()[i * 128:(i + 1) * 128, :])
                # fill brep/crep via partition-broadcast DMA from DRAM
                for j in range(2 * N):
                    rep = brep[j] if j < N else crep[j - N]
                    nc.sync.dma_start(
                        rep[:],
                        dbl_dram[j:j + 1, :].broadcast_to([128, L]))

                # ============ phase SC: selective scan over n ============
                with (
                    tc.tile_pool(name="scan", bufs=2) as scanp,
                    tc.tile_pool(name="psy", bufs=1, space="PSUM") as psy,
                    tc.tile_pool(name="psx", bufs=2, space="PSUM") as psx,
                    tc.tile_pool(name="stg", bufs=3) as stgp,
                ):
                    first_group = True
                    uidx = 0
                    for mts in ((0, 1, 2), (3, 4, 5), (6, 7)):
                        yps = {mt: psy.tile([128, L], FP32, name=f"psy{j}",
                                            tag=f"psy{j}")
                               for j, mt in enumerate(mts)}
                        for mt in mts:
                            for h in range(2):
                                nc.tensor.matmul(
                                    yps[mt][:, h * 512:(h + 1) * 512],
                                    ident_sb[:],
                                    y_acc[mt][:, h * 512:(h + 1) * 512],
                                    start=True, stop=False)
                        for n in range(N):
                            for mt in mts:
                                dA = scanp.tile([128, L], BF16, name="dA",
                                                tag="dA")
                                nc.scalar.activation(
                                    dA[:], deltaT[mt][:], AF.Exp,
                                    scale=a_sb[mt][:, n:n + 1])
                                dBx = scanp.tile([128, L], BF16, name="dBx",
                                                 tag="dBx")
                                nc.vector.tensor_tensor(dBx[:], w_t[mt][:],
                                                        brep[n][:], OP.mult)
                                hh = scanp.tile([128, L], BF16, name="hh",
                                                tag="hh", bufs=4)
                                nc.vector.tensor_tensor_scan(
                                    hh[:], dA[:], dBx[:], 0.0, OP.mult, OP.add)
                                prod = scanp.tile([128, L], BF16, name="prod",
                                                  tag="prod", bufs=4)
                                use_pool = _POOL_PAT[uidx % 8]
                                uidx += 1
                                peng = nc.gpsimd if use_pool else nc.vector
                                peng.tensor_tensor(prod[:], hh[:], crep[n][:],
                                                   OP.mult)
                                for h in range(2):
                                    nc.tensor.matmul(
                                        yps[mt][:, h * 512:(h + 1) * 512],
                                        ident_sb[:],
                                        prod[:, h * 512:(h + 1) * 512],
                                        start=False, stop=(n == N - 1))
                        for mt in mts:
                            nc.scalar.copy(y_acc[mt][:], yps[mt][:])
                            # gate: yf = y * silu(z), into the dead xp tile
                            nc.vector.tensor_tensor(xp_bf[mt][:], y_acc[mt][:],
                                                    silu_z[mt][:], OP.mult)
                        if first_group:
                            first_group = False
                            # x-term gating partial: x @ W-half (PE slack)
                            for mo2 in range(NMD):
                                for h in range(2):
                                    psq = psx.tile([128, 512], FP32,
                                                   name="psq", tag="psq")
                                    for mo in range(NMD):
                                        nc.tensor.matmul(
                                            psq[:],
                                            wxh[mo][:,
                                                    mo2 * 128:(mo2 + 1) * 128],
                                            xbf[mo][:, h * 512:(h + 1) * 512],
                                            start=(mo == 0),
                                            stop=(mo == NMD - 1))
                                    sq = stgp.tile([128, 512], FP32,
                                                   name="sq", tag="sq")
                                    nc.scalar.copy(sq[:], psq[:])
                                    nc.sync.dma_start(
                                        pre_x.ap()[mo2 * 128:(mo2 + 1) * 128,
                                                   h * 512:(h + 1) * 512],
                                        sq[:])

            # ============ phase FN: out_proj + partial gating ============
            with (
                tc.tile_pool(name="fin", bufs=1) as finp,
                tc.tile_pool(name="pso", bufs=4, space="PSUM") as pso_p,
                tc.tile_pool(name="stg2", bufs=3) as stg2,
            ):
                wo_sb = [finp.tile([128, Dm], BF16, name=f"wo{i}",
                                   tag=f"wo{i}")
                         for i in range(NSH)]
                for i in range(NSH):
                    nc.sync.dma_start(wo_sb[i][:],
                                      w_out.ap()[i * 128:(i + 1) * 128, :])
                wgv = [finp.tile([128, Dm], BF16, name=f"wgv{i}",
                                 tag=f"wgv{i}")
                       for i in range(2 * NMD)]
                for i in range(NMD):
                    nc.sync.dma_start(wgv[i][:],
                                      wg_h.ap()[i * 128:(i + 1) * 128, :])
                    nc.sync.dma_start(wgv[NMD + i][:],
                                      wv_h.ap()[i * 128:(i + 1) * 128, :])
                pbf = [finp.tile([128, L], BF16, name=f"pb{i}", tag=f"pb{i}")
                       for i in range(NMD)]
                for mo in range(NMD):
                    for h in range(2):
                        pso = pso_p.tile([128, 512], FP32, name="pso",
                                         tag="pso")
                        for kt in range(NSH):
                            nc.tensor.matmul(
                                pso[:], wo_sb[kt][:, mo * 128:(mo + 1) * 128],
                                xp_bf[kt][:, h * 512:(h + 1) * 512],
                                start=(kt == 0), stop=(kt == NSH - 1))
                        nc.scalar.copy(pbf[mo][:, h * 512:(h + 1) * 512],
                                       pso[:])
                    nc.sync.dma_start(p_out.ap()[mo * 128:(mo + 1) * 128, :],
                                      pbf[mo][:])
                # partial gating: own p @ Wg-half and @ Wv-half
                for gv in range(2):
                    dst = pre_a if gv == 0 else pre_b
                    for mo2 in range(NMD):
                        for h in range(2):
                            psg = pso_p.tile([128, 512], FP32, name="psg",
                                             tag="psg")
                            for mo in range(NMD):
                                nc.tensor.matmul(
                                    psg[:],
                                    wgv[gv * NMD + mo][
                                        :, mo2 * 128:(mo2 + 1) * 128],
                                    pbf[mo][:, h * 512:(h + 1) * 512],
                                    start=(mo == 0), stop=(mo == NMD - 1))
                            sg2 = stg2.tile([128, 512], FP32,
                                            name="sg2", tag="sg2")
                            nc.scalar.copy(sg2[:], psg[:])
                            nc.sync.dma_start(
                                dst.ap()[mo2 * 128:(mo2 + 1) * 128,
                                         h * 512:(h + 1) * 512],
                                sg2[:])

    nc.compile()
    return nc


# ------------------------------------------------------------------- host
_cache = {}


def _get_nc(which=1):
    if which not in _cache:
        _cache[which] = build_launch1()
    return _cache[which]


def prep_launch1_inmaps(x, ln_w, ln_b, W_in, b_in, conv_w, conv_b, W_xproj,
                        W_dt, b_dt, A_log, D, W_out, b_out, Wg, Wv):
    """Build the 8 per-core input dicts."""
    in_maps = []
    ident_np = np.eye(128, dtype=np.float32).astype(BF)
    xf = [np.ascontiguousarray(x[b].T) for b in range(B)]           # [Dm, L]
    xr = [np.ascontiguousarray(x[b, ::-1].T) for b in range(B)]     # reversed
    wg_bf = [Wg[dr * Dm:(dr + 1) * Dm, :].astype(BF) for dr in range(2)]
    wv_bf = [Wv[dr * Dm:(dr + 1) * Dm, :].astype(BF) for dr in range(2)]
    for core in range(NCORES):
        b, dr, s = core >> 2, (core >> 1) & 1, core & 1
        sl = slice(s * DiS, (s + 1) * DiS)
        W_eff = ln_w[dr][:, None] * W_in[dr]                         # [Dm, 4096]
        b_eff = ln_b[dr] @ W_in[dr] + b_in[dr]                       # [4096]
        # xp channel permutation: this core's Di-shard channels come first
        perm = np.concatenate([np.arange(s * DiS, (s + 1) * DiS),
                               np.arange((1 - s) * DiS, (2 - s) * DiS)])
        cols = np.concatenate([perm, Di + s * DiS + np.arange(DiS)])
        Wc = W_eff[:, cols]                                          # [Dm, 3072]
        in_maps.append({
            "xT": (xf if dr == 0 else xr)[b],
            "w_in": Wc.astype(BF),
            "w_in_c": (-Wc.sum(0, keepdims=True)).astype(BF),
            "b_in": b_eff[cols][:, None].astype(np.float32),
            "conv_w": conv_w[dr][perm].astype(np.float32),
            "conv_b": conv_b[dr][perm][:, None].astype(np.float32),
            "w_xp": W_xproj[dr][perm].astype(BF),
            "w_dt": W_dt[dr][:, sl].astype(BF),
            "b_dt": b_dt[dr][sl][:, None].astype(np.float32),
            "a_mat": (-np.exp(A_log[dr][sl])).astype(np.float32),
            "d_vec": D[dr][sl][:, None].astype(np.float32),
            "w_out": W_out[dr][sl, :].astype(BF),
            "wg_h": wg_bf[dr],
            "wv_h": wv_bf[dr],
            "wx_h": wg_bf[dr] if s == 0 else wv_bf[dr],
            "ident": ident_np,
        })
    return in_maps, xf


def postprocess(res1, x, b_out, Wg, bg, Wv, bv):
    """Host combine: sums of partials, sigmoid gate, convex blend."""
    idx = lambda b, dr, s: (b << 2) | (dr << 1) | s
    out = np.empty((B, L, Dm), np.float32)
    f32 = np.float32
    bias_g = (b_out[0] @ Wg[:Dm] + b_out[1] @ Wg[Dm:] + bg).astype(f32)
    bias_v = (b_out[0] @ Wv[:Dm] + b_out[1] @ Wv[Dm:] + bv).astype(f32)
    for b in range(B):
        pf = (res1[idx(b, 0, 0)]["p_out"].astype(f32)
              + res1[idx(b, 0, 1)]["p_out"].astype(f32))
        pb = (res1[idx(b, 1, 0)]["p_out"].astype(f32)
              + res1[idx(b, 1, 1)]["p_out"].astype(f32))[:, ::-1]
        xb = x[b].T.astype(f32)                               # [Dm, L]
        fwd = xb + b_out[0][:, None] + pf
        bwd = xb + b_out[1][:, None] + pb
        s_sum = fwd + bwd
        pre_g = (res1[idx(b, 0, 0)]["pre_a"] + res1[idx(b, 0, 1)]["pre_a"]
                 + res1[idx(b, 1, 0)]["pre_a"][:, ::-1]
                 + res1[idx(b, 1, 1)]["pre_a"][:, ::-1]
                 + res1[idx(b, 0, 0)]["pre_x"]
                 + res1[idx(b, 1, 0)]["pre_x"][:, ::-1])
        pre_v = (res1[idx(b, 0, 0)]["pre_b"] + res1[idx(b, 0, 1)]["pre_b"]
                 + res1[idx(b, 1, 0)]["pre_b"][:, ::-1]
                 + res1[idx(b, 1, 1)]["pre_b"][:, ::-1]
                 + res1[idx(b, 0, 1)]["pre_x"]
                 + res1[idx(b, 1, 1)]["pre_x"][:, ::-1])
        g = 1.0 / (1.0 + np.exp(-(pre_g + bias_g[:, None])))
        v = pre_v + bias_v[:, None]
        out[b] = (0.5 * (g * (v - s_sum) + s_sum)).T
    return out


def kernel(x, ln_w, ln_b, W_in, b_in, conv_w, conv_b, W_xproj, W_dt, b_dt,
           A_log, D, W_out, b_out, Wg, bg, Wv, bv):
    x = np.asarray(x, np.float32)
    args = [np.asarray(a, np.float32) for a in
            (ln_w, ln_b, W_in, b_in, conv_w, conv_b, W_xproj, W_dt, b_dt,
             A_log, D, W_out, b_out)]
    Wg, bg, Wv, bv = (np.asarray(a, np.float32) for a in (Wg, bg, Wv, bv))

    in1, xf = prep_launch1_inmaps(x, *args, Wg, Wv)
    nc1 = _get_nc(1)
    res1 = run_bass_kernel_spmd(nc1, in1, core_ids=list(range(NCORES))).results
    return postprocess(res1, x, args[-1], Wg, bg, Wv, bv)


# revision 18
# speedup vs baseline: 1.2798x; 1.0014x over previous
"""BiMambaBlock Trainium2 kernel (8-core SPMD via Bass/Tile), single launch.

Sharding: core = (b, dir, s) with b in {0,1} batch, dir in {fwd, bwd},
s in {0,1} half of d_inner (2048 -> 1024 per core).

Per core: layernorm (folded into W_in) -> in_proj -> causal depthwise
conv + silu -> x_proj -> dt_proj/softplus -> selective scan
(tensor_tensor_scan over time per (state n, 128-channel block); dBx on
DVE, prod on GPSIMD) -> gate by silu(z) -> out_proj partial -> partial
gating matmuls (own p @ Wg/Wv half, plus x @ per-core W half) for the
final bi-directional combine, which the host finishes (partial sums,
sigmoid, convex blend -- elementwise only, no host matmuls).

Everything on-device operates in time-transposed layout [feature, L].
bwd direction runs on host-reversed time; host un-reverses partials.
"""

import os
import sys

sys.path.insert(0, "/opt/trn_rl_repo")

SIM_SAFE = bool(os.environ.get("KERNEL_SIM_SAFE"))
# Fraction (in eighths) of the prod (h*C) multiplies run on GPSIMD to
# free DVE (the scan-phase bottleneck). GPSIMD under SBUF contention runs
# ~4-5us per [128,1024] op, so it can only absorb ~60%.
POOL_EIGHTHS = int(os.environ.get("KERNEL_POOL_EIGHTHS", "0"))
_POOL_PAT = [i * POOL_EIGHTHS % 8 < POOL_EIGHTHS for i in range(8)]
_POOL_PAT = ([True] * POOL_EIGHTHS + [False] * (8 - POOL_EIGHTHS))

import numpy as np
import ml_dtypes

import concourse.bass as bass
import concourse.mybir as mybir
import concourse.tile as tile
from concourse import bacc
from concourse.bass_utils import run_bass_kernel_spmd

FP32 = mybir.dt.float32
BF16 = mybir.dt.bfloat16
AF = mybir.ActivationFunctionType
OP = mybir.AluOpType
BF = ml_dtypes.bfloat16

B, L, Dm, Di, N, R, KC = 2, 1024, 1024, 2048, 16, 64, 4
DiS = Di // 2  # 1024 channels per core
EPS = 1e-5
NCORES = 8

NXP = Di // 128        # 16 xp channel tiles
NSH = DiS // 128       # 8 shard channel tiles
NMD = Dm // 128        # 8 model-dim tiles


def build_launch1():
    nc = bacc.Bacc("TRN2", target_bir_lowering=False, debug=False,
                   num_devices=NCORES)
    xT = nc.dram_tensor("xT", [Dm, L], FP32, kind="ExternalInput")
    w_in = nc.dram_tensor("w_in", [Dm, 3072], BF16, kind="ExternalInput")
    w_in_c = nc.dram_tensor("w_in_c", [1, 3072], BF16, kind="ExternalInput")
    b_in = nc.dram_tensor("b_in", [3072, 1], FP32, kind="ExternalInput")
    conv_w = nc.dram_tensor("conv_w", [Di, KC], FP32, kind="ExternalInput")
    conv_b = nc.dram_tensor("conv_b", [Di, 1], FP32, kind="ExternalInput")
    w_xp = nc.dram_tensor("w_xp", [Di, 96], BF16, kind="ExternalInput")
    w_dt = nc.dram_tensor("w_dt", [R, DiS], BF16, kind="ExternalInput")
    b_dt = nc.dram_tensor("b_dt", [DiS, 1], FP32, kind="ExternalInput")
    a_mat = nc.dram_tensor("a_mat", [DiS, N], FP32, kind="ExternalInput")
    d_vec = nc.dram_tensor("d_vec", [DiS, 1], FP32, kind="ExternalInput")
    w_out = nc.dram_tensor("w_out", [DiS, Dm], BF16, kind="ExternalInput")
    wg_h = nc.dram_tensor("wg_h", [Dm, Dm], BF16, kind="ExternalInput")
    wv_h = nc.dram_tensor("wv_h", [Dm, Dm], BF16, kind="ExternalInput")
    wx_h = nc.dram_tensor("wx_h", [Dm, Dm], BF16, kind="ExternalInput")
    ident = nc.dram_tensor("ident", [128, 128], BF16, kind="ExternalInput")
    w_fg = nc.dram_tensor("w_fg", [DiS, Dm], BF16, kind="ExternalInput")
    w_fv = nc.dram_tensor("w_fv", [DiS, Dm], BF16, kind="ExternalInput")
    p_out = nc.dram_tensor("p_out", [3, Dm, L], BF16, kind="ExternalOutput")
    pre_a = nc.dram_tensor("pre_a", [3, Dm, L], FP32, kind="ExternalOutput")
    pre_b = nc.dram_tensor("pre_b", [3, Dm, L], FP32, kind="ExternalOutput")
    pre_x = nc.dram_tensor("pre_x", [Dm, L], FP32, kind="ExternalOutput")

    with tile.TileContext(nc) as tc:
        with (
            tc.tile_pool(name="pers", bufs=1) as pers,
            tc.tile_pool(name="bias", bufs=2) as biasp,
            tc.tile_pool(name="dsc", bufs=1, space="DRAM") as dscp,
        ):
            dbl_dram = dscp.tile([2 * N, L], BF16, name="dbldr", tag="dbldr")
            # --- persistent tiles (whole-kernel lifetime): 14 MB ---
            silu_z = [pers.tile([128, L], BF16, name=f"sz{i}", tag=f"sz{i}")
                      for i in range(NSH)]
            xp_bf = [pers.tile([128, L], BF16, name=f"xp{i}", tag=f"xp{i}")
                     for i in range(NSH)]
            deltaT = [pers.tile([128, L], BF16, name=f"dl{i}", tag=f"dl{i}")
                      for i in range(NSH)]
            w_t = [pers.tile([128, L], BF16, name=f"wt{i}", tag=f"wt{i}")
                   for i in range(NSH)]
            y_acc = [pers.tile([128, L], BF16, name=f"ya{i}", tag=f"ya{i}")
                     for i in range(NSH)]
            xbf = [pers.tile([128, L], BF16, name=f"xb{i}", tag=f"xb{i}")
                   for i in range(NMD)]
            a_sb = [pers.tile([128, N], FP32, name=f"a{i}", tag=f"a{i}")
                    for i in range(NSH)]
            d_sb = [pers.tile([128, 1], FP32, name=f"d{i}", tag=f"d{i}")
                    for i in range(NSH)]
            ident_sb = pers.tile([128, 128], BF16, name="identsb",
                                 tag="identsb")
            nc.sync.dma_start(ident_sb[:], ident.ap())
            ones_f = pers.tile([1, 128], FP32, name="onesf", tag="onesf")
            ones_r = pers.tile([128, 1], FP32, name="onesr", tag="onesr")
            nc.vector.memset(ones_f[:], 1.0)
            nc.vector.memset(ones_r[:], 1.0)
            for i in range(NSH):
                nc.sync.dma_start(a_sb[i][:], a_mat.ap()[i * 128:(i + 1) * 128, :])
                nc.sync.dma_start(d_sb[i][:], d_vec.ap()[i * 128:(i + 1) * 128, :])

            # xp_hi: channels of the other Di half (x_proj input only)
            with tc.tile_pool(name="xph", bufs=1) as xph:
              xp_hi = [xph.tile([128, L], BF16, name=f"xh{i}", tag=f"xh{i}")
                       for i in range(NXP - NSH)]
              dbl_bf = xph.tile([96, L], BF16, name="dbl", tag="dbl")
              wx = [xph.tile([128, 96], BF16, name=f"wx{i}", tag=f"wx{i}")
                    for i in range(NXP)]
              for i in range(NXP):
                  nc.sync.dma_start(wx[i][:], w_xp.ap()[i * 128:(i + 1) * 128, :])
              wdt = xph.tile([R, DiS], BF16, name="wdt", tag="wdt")
              nc.sync.dma_start(wdt[:], w_dt.ap())

              # ============ phase IP: LN stats + z1 + in_proj ============
              with (
                tc.tile_pool(name="ip", bufs=1) as ip,
                tc.tile_pool(name="wks", bufs=3) as wks,
                tc.tile_pool(name="cvt", bufs=2) as cvt,
              ):
                z1 = [ip.tile([128, L], BF16, name=f"z1{i}", tag=f"z1{i}")
                      for i in range(NMD)]
                mu = ip.tile([1, L], FP32, name="mu", tag="mu")
                rstd = ip.tile([1, L], FP32, name="rstd", tag="rstd")
                mr_row = ip.tile([1, L], BF16, name="mr", tag="mr")
                rstd_b = ip.tile([128, L], BF16, name="rstdb", tag="rstdb")
                wc = ip.tile([1, 3072], BF16, name="wc", tag="wc")
                nc.sync.dma_start(wc[:], w_in_c.ap())

                # ---- pass 1: stats ----
                with (
                    tc.tile_pool(name="sta", bufs=2) as sta,
                    tc.tile_pool(name="psst", bufs=1, space="PSUM") as psst,
                ):
                    ps_mu = psst.tile([1, L], FP32, name="psmu", tag="psmu")
                    ps_sq = psst.tile([1, L], FP32, name="pssq", tag="pssq")
                    for i in range(NMD):
                        xti = sta.tile([128, L], FP32, name="xti", tag="xti",
                                       bufs=3)
                        nc.sync.dma_start(xti[:],
                                          xT.ap()[i * 128:(i + 1) * 128, :])
                        x2i = sta.tile([128, L], FP32, name="x2i", tag="x2i",
                                       bufs=1)
                        nc.scalar.activation(x2i[:], xti[:], AF.Square)
                        nc.vector.tensor_copy(xbf[i][:], xti[:])
                        for h in range(2):
                            sl = slice(h * 512, (h + 1) * 512)
                            nc.tensor.matmul(ps_mu[:, sl], ones_r[:],
                                             xti[:, sl],
                                             start=(i == 0), stop=(i == NMD - 1))
                            nc.tensor.matmul(ps_sq[:, sl], ones_r[:], x2i[:, sl],
                                             start=(i == 0), stop=(i == NMD - 1))
                    nc.scalar.mul(mu[:], ps_mu[:], 1.0 / Dm)
                    msq = sta.tile([1, L], FP32, name="strow", tag="strow", bufs=3)
                    nc.scalar.mul(msq[:], ps_sq[:], 1.0 / Dm)
                    mu2 = sta.tile([1, L], FP32, name="strow", tag="strow", bufs=3)
                    nc.vector.tensor_tensor(mu2[:], mu[:], mu[:], OP.mult)
                    var = sta.tile([1, L], FP32, name="strow", tag="strow", bufs=3)
                    nc.vector.tensor_tensor(var[:], msq[:], mu2[:], OP.subtract)
                    eps_t = sta.tile([1, 1], FP32, name="epst", tag="epst")
                    nc.vector.memset(eps_t[:], EPS)
                    lnv = sta.tile([1, L], FP32, name="strow", tag="strow",
                                   bufs=3)
                    nc.scalar.activation(lnv[:], var[:], AF.Ln, bias=eps_t[:])
                    nc.scalar.activation(rstd[:], lnv[:], AF.Exp, scale=-0.5)
                    nc.vector.tensor_tensor(mr_row[:], mu[:], rstd[:], OP.mult)
                    for h in range(2):
                        psb = psst.tile([128, 512], FP32, name="psb0", tag="psb0")
                        nc.tensor.matmul(psb[:], ones_f[:],
                                         rstd[:, h * 512:(h + 1) * 512],
                                         start=True, stop=True)
                        nc.scalar.copy(rstd_b[:, h * 512:(h + 1) * 512], psb[:])
                    # ---- pass 2: z1 = bf16(xT) * rstd (no re-DMA) ----
                    for i in range(NMD):
                        nc.vector.tensor_tensor(z1[i][:], xbf[i][:],
                                                rstd_b[:], OP.mult)

                # ---- in_proj: groups of 2 output tiles, stream weights ----
                # xp tiles (mt 0..15) first, then xproj/dt interlude, then z.
                def in_proj_group(mts2, psin):
                    pst = [psin.tile([128, 512], FP32, name="psi",
                                     tag="psi", bufs=4) for _ in range(4)]
                    for kt in range(NMD):
                        wkt = wks.tile([128, 256], BF16, name="wkt",
                                       tag="wkt")
                        nc.sync.dma_start(
                            wkt[:],
                            w_in.ap()[kt * 128:(kt + 1) * 128,
                                      mts2[0] * 128:(mts2[0] + 2) * 128])
                        for m2 in range(2):
                            for h in range(2):
                                nc.tensor.matmul(
                                    pst[m2 * 2 + h][:],
                                    wkt[:, m2 * 128:(m2 + 1) * 128],
                                    z1[kt][:, h * 512:(h + 1) * 512],
                                    start=(kt == 0), stop=False)
                    for m2, mt in enumerate(mts2):
                        for h in range(2):
                            nc.tensor.matmul(
                                pst[m2 * 2 + h][:],
                                wc[:, mt * 128:(mt + 1) * 128],
                                mr_row[:, h * 512:(h + 1) * 512],
                                start=False, stop=True)
                    for m2, mt in enumerate(mts2):
                        bi = biasp.tile([128, 1], FP32, name="bin",
                                        tag="bin")
                        nc.sync.dma_start(
                            bi[:], b_in.ap()[mt * 128:(mt + 1) * 128, :])
                        if mt < NXP:
                            # causal conv: 4 psum-accumulated diag matmuls
                            xpad = cvt.tile([128, L + 3], BF16,
                                            name="xpad", tag="xpad")
                            nc.vector.memset(xpad[:, 0:3], 0.0)
                            for h in range(2):
                                nc.scalar.activation(
                                    xpad[:, 3 + h * 512:3 + (h + 1) * 512],
                                    pst[m2 * 2 + h][:], AF.Identity,
                                    bias=bi[:])
                            cw = biasp.tile([128, KC], FP32, name="cw",
                                            tag="cw")
                            cb = biasp.tile([128, 1], FP32, name="cb",
                                            tag="cb")
                            nc.sync.dma_start(
                                cw[:], conv_w.ap()[mt * 128:(mt + 1) * 128, :])
                            nc.sync.dma_start(
                                cb[:], conv_b.ap()[mt * 128:(mt + 1) * 128, :])
                            diags = []
                            for j in range(KC):
                                dg = cvt.tile([128, 128], BF16,
                                              name="diag", tag="diag",
                                              bufs=8)
                                nc.vector.tensor_scalar_mul(
                                    dg[:], ident_sb[:], cw[:, j:j + 1])
                                diags.append(dg)
                            dst = (xp_bf[mt] if mt < NSH
                                   else xp_hi[mt - NSH])
                            for h in range(2):
                                pcv = psin.tile([128, 512], FP32,
                                                name="pcv", tag="pcv",
                                                bufs=2)
                                for j in range(KC):
                                    nc.tensor.matmul(
                                        pcv[:], diags[j][:],
                                        xpad[:, j + h * 512:
                                             j + h * 512 + 512],
                                        start=(j == 0), stop=(j == KC - 1))
                                cs = slice(h * 512, (h + 1) * 512)
                                if SIM_SAFE:
                                    sg = cvt.tile([128, 512], BF16,
                                                  name="sg", tag="sg")
                                    nc.scalar.activation(
                                        sg[:], pcv[:], AF.Sigmoid,
                                        bias=cb[:])
                                    zz = cvt.tile([128, 512], BF16,
                                                  name="zz", tag="zz")
                                    nc.scalar.activation(
                                        zz[:], pcv[:], AF.Identity,
                                        bias=cb[:])
                                    nc.vector.tensor_tensor(
                                        dst[:, cs], zz[:], sg[:],
                                        OP.mult)
                                else:
                                    nc.scalar.activation(
                                        dst[:, cs], pcv[:],
                                        AF.Silu, bias=cb[:])
                        else:
                            zt = silu_z[mt - NXP]
                            for h in range(2):
                                cs = slice(h * 512, (h + 1) * 512)
                                if SIM_SAFE:
                                    sg = cvt.tile([128, 512], BF16,
                                                  name="sg2", tag="sg2")
                                    nc.scalar.activation(
                                        sg[:], pst[m2 * 2 + h][:],
                                        AF.Sigmoid, bias=bi[:])
                                    zz = cvt.tile([128, 512], BF16,
                                                  name="zz2", tag="zz2")
                                    nc.scalar.activation(
                                        zz[:], pst[m2 * 2 + h][:],
                                        AF.Identity, bias=bi[:])
                                    nc.vector.tensor_tensor(
                                        zt[:, cs], zz[:], sg[:], OP.mult)
                                else:
                                    nc.scalar.activation(
                                        zt[:, cs],
                                        pst[m2 * 2 + h][:], AF.Silu,
                                        bias=bi[:])

                with tc.tile_pool(name="psin", bufs=4, space="PSUM") as psin:
                    for g in range(8):            # xp tiles mt 0..15
                        in_proj_group((2 * g, 2 * g + 1), psin)

                    # ---- xproj (needs all xp) ----
                    for h in range(2):
                        psda = psin.tile([128, 512], FP32, name="aux",
                                         tag="aux", bufs=2)
                        psd = psda[0:96, :]
                        for kt in range(NXP):
                            srct = xp_bf[kt] if kt < NSH else xp_hi[kt - NSH]
                            nc.tensor.matmul(psd[:], wx[kt][:],
                                             srct[:, h * 512:(h + 1) * 512],
                                             start=(kt == 0),
                                             stop=(kt == NXP - 1))
                        nc.scalar.copy(dbl_bf[:, h * 512:(h + 1) * 512],
                                       psd[:])
                    # bounce B/C rows through DRAM (broadcast reload below)
                    nc.sync.dma_start(dbl_dram[:], dbl_bf[R:R + 2 * N, :])

                    # ---- dt_proj + softplus + w/y-init (cheap PE, long
                    # ACT chain), then z-half in_proj overlaps the chain ----
                    for blk in range(4):
                        if False:
                            pass
                        for mt in (2 * blk, 2 * blk + 1):
                            bdt = biasp.tile([128, 1], FP32, name="bdt",
                                             tag="bdt")
                            nc.sync.dma_start(
                                bdt[:], b_dt.ap()[mt * 128:(mt + 1) * 128, :])
                            for h in range(2):
                                psdt = psin.tile([128, 512], FP32,
                                                 name="aux", tag="aux",
                                                 bufs=2)
                                nc.tensor.matmul(
                                    psdt[:], wdt[:, mt * 128:(mt + 1) * 128],
                                    dbl_bf[0:R, h * 512:(h + 1) * 512],
                                    start=True, stop=True)
                                # softplus(x) = ln(1 + exp(x)); x << 80
                                edt = cvt.tile([128, 512], FP32, name="edt",
                                               tag="edt", bufs=4)
                                nc.scalar.activation(edt[:], psdt[:], AF.Exp,
                                                     bias=bdt[:])
                                nc.scalar.activation(
                                    deltaT[mt][:, h * 512:(h + 1) * 512],
                                    edt[:], AF.Ln, bias=ones_r[:, 0:1])
                            nc.vector.tensor_tensor(w_t[mt][:],
                                                    deltaT[mt][:],
                                                    xp_bf[mt][:], OP.mult)
                            nc.vector.tensor_scalar_mul(y_acc[mt][:],
                                                        xp_bf[mt][:],
                                                        d_sb[mt][:])
                    for blk in range(4):
                        in_proj_group((16 + 2 * blk, 17 + 2 * blk), psin)

            # ======= B/C broadcast hoist + scan, then FN =======
            with tc.tile_pool(name="bcp", bufs=1) as bcp:
                brep = [bcp.tile([128, L], BF16, name=f"br{j}", tag=f"br{j}")
                        for j in range(8)]
                crep = [bcp.tile([128, L], BF16, name=f"cr{j}", tag=f"cr{j}")
                        for j in range(8)]
                wxh = [bcp.tile([128, Dm], BF16, name=f"wxh{i}",
                                tag=f"wxh{i}")
                       for i in range(NMD)]
                for i in range(NMD):
                    nc.sync.dma_start(wxh[i][:],
                                      wx_h.ap()[i * 128:(i + 1) * 128, :])

                def load_rep_half(nh):
                    # (re)load b/c rows for n in [8*nh, 8*nh+8) via
                    # partition-broadcast DMA; tiles are reused mod 8
                    for nn in range(8 * nh, 8 * nh + 8):
                        nc.sync.dma_start(
                            brep[nn % 8][:],
                            dbl_dram[nn:nn + 1, :].broadcast_to([128, L]))
                        nc.sync.dma_start(
                            crep[nn % 8][:],
                            dbl_dram[N + nn:N + nn + 1,
                                     :].broadcast_to([128, L]))

                # ============ phase SC: selective scan over n ============
                with (
                    tc.tile_pool(name="scan", bufs=2) as scanp,
                    tc.tile_pool(name="psy", bufs=1, space="PSUM") as psy,
                    tc.tile_pool(name="psx", bufs=2, space="PSUM") as psx,
                    tc.tile_pool(name="stg", bufs=3) as stgp,
                    tc.tile_pool(name="wfn", bufs=1) as wfnp,
                ):
                    first_group = True
                    uidx = 0
                    for gi, mts in enumerate(((0, 1, 2), (3, 4, 5), (6, 7))):
                        yps = {mt: psy.tile([128, L], FP32, name=f"psy{j}",
                                            tag=f"psy{j}")
                               for j, mt in enumerate(mts)}
                        for mt in mts:
                            for h in range(2):
                                nc.tensor.matmul(
                                    yps[mt][:, h * 512:(h + 1) * 512],
                                    ident_sb[:],
                                    y_acc[mt][:, h * 512:(h + 1) * 512],
                                    start=True, stop=False)
                        # prefetch this group's fused FN weight rows
                        wrows = {}
                        for si, wsrc in enumerate((w_out, w_fg, w_fv)):
                            for mt in mts:
                                wt_ = wfnp.tile([128, Dm], BF16, name="wfn",
                                                tag=f"wf{si}{mt % 3}")
                                nc.sync.dma_start(
                                    wt_[:],
                                    wsrc.ap()[mt * 128:(mt + 1) * 128, :])
                                wrows[(si, mt)] = wt_
                        for n in range(N):
                            if n % 8 == 0:
                                load_rep_half(n // 8)
                            for mt in mts:
                                dA = scanp.tile([128, L], BF16, name="dA",
                                                tag="dA")
                                nc.scalar.activation(
                                    dA[:], deltaT[mt][:], AF.Exp,
                                    scale=a_sb[mt][:, n:n + 1])
                                dBx = scanp.tile([128, L], BF16, name="dBx",
                                                 tag="dBx")
                                nc.vector.tensor_tensor(dBx[:], w_t[mt][:],
                                                        brep[n % 8][:],
                                                        OP.mult)
                                hh = scanp.tile([128, L], BF16, name="hh",
                                                tag="hh", bufs=4)
                                nc.vector.tensor_tensor_scan(
                                    hh[:], dA[:], dBx[:], 0.0, OP.mult, OP.add)
                                prod = scanp.tile([128, L], BF16, name="prod",
                                                  tag="prod", bufs=4)
                                use_pool = (POOL_EIGHTHS > 0
                                            and _POOL_PAT[uidx % 8])
                                uidx += 1
                                peng = nc.gpsimd if use_pool else nc.vector
                                peng.tensor_tensor(prod[:], hh[:],
                                                   crep[n % 8][:], OP.mult)
                                for h in range(2):
                                    nc.tensor.matmul(
                                        yps[mt][:, h * 512:(h + 1) * 512],
                                        ident_sb[:],
                                        prod[:, h * 512:(h + 1) * 512],
                                        start=False, stop=(n == N - 1))
                        for mt in mts:
                            nc.scalar.copy(y_acc[mt][:], yps[mt][:])
                            # gate: yf = y * silu(z), into the dead xp tile
                            nc.vector.tensor_tensor(xp_bf[mt][:], y_acc[mt][:],
                                                    silu_z[mt][:], OP.mult)
                        # fused partial FN for this group:
                        #   set 0: p_part    = yf[g] @ W_out[g-rows]
                        #   set 1: prea_part = yf[g] @ (W_out @ Wg_h)[g-rows]
                        #   set 2: preb_part = yf[g] @ (W_out @ Wv_h)[g-rows]
                        for si, dst in enumerate((p_out, pre_a, pre_b)):
                            for mo in range(NMD):
                                for h in range(2):
                                    psq = psx.tile([128, 512], FP32,
                                                   name="psq", tag="psq")
                                    for ki, kt in enumerate(mts):
                                        nc.tensor.matmul(
                                            psq[:],
                                            wrows[(si, kt)][
                                                :, mo * 128:(mo + 1) * 128],
                                            xp_bf[kt][:,
                                                      h * 512:(h + 1) * 512],
                                            start=(ki == 0),
                                            stop=(ki == len(mts) - 1))
                                    sq = stgp.tile(
                                        [128, 512],
                                        BF16 if si == 0 else FP32,
                                        name="sq",
                                        tag="sqp" if si == 0 else "sqf",
                                        bufs=3)
                                    nc.scalar.copy(sq[:], psq[:])
                                    nc.sync.dma_start(
                                        dst.ap()[gi,
                                                 mo * 128:(mo + 1) * 128,
                                                 h * 512:(h + 1) * 512],
                                        sq[:])
                        if first_group:
                            first_group = False
                            # x-term gating partial: x @ W-half (PE slack)
                            for mo2 in range(NMD):
                                for h in range(2):
                                    psq = psx.tile([128, 512], FP32,
                                                   name="psq", tag="psq")
                                    for mo in range(NMD):
                                        nc.tensor.matmul(
                                            psq[:],
                                            wxh[mo][:,
                                                    mo2 * 128:(mo2 + 1) * 128],
                                            xbf[mo][:, h * 512:(h + 1) * 512],
                                            start=(mo == 0),
                                            stop=(mo == NMD - 1))
                                    sq = stgp.tile([128, 512], FP32,
                                                   name="sq", tag="sqf",
                                                   bufs=3)
                                    nc.scalar.copy(sq[:], psq[:])
                                    nc.sync.dma_start(
                                        pre_x.ap()[mo2 * 128:(mo2 + 1) * 128,
                                                   h * 512:(h + 1) * 512],
                                        sq[:])

    nc.compile()
    return nc


# ------------------------------------------------------------------- host
_cache = {}


def _get_nc(which=1):
    if which not in _cache:
        _cache[which] = build_launch1()
    return _cache[which]


def prep_launch1_inmaps(x, ln_w, ln_b, W_in, b_in, conv_w, conv_b, W_xproj,
                        W_dt, b_dt, A_log, D, W_out, b_out, Wg, Wv):
    """Build the 8 per-core input dicts."""
    in_maps = []
    ident_np = np.eye(128, dtype=np.float32).astype(BF)
    xf = [np.ascontiguousarray(x[b].T) for b in range(B)]           # [Dm, L]
    xr = [np.ascontiguousarray(x[b, ::-1].T) for b in range(B)]     # reversed
    wg_bf = [Wg[dr * Dm:(dr + 1) * Dm, :].astype(BF) for dr in range(2)]
    wv_bf = [Wv[dr * Dm:(dr + 1) * Dm, :].astype(BF) for dr in range(2)]
    # fused out_proj+gating weights (host matmul of weights only)
    wfg = [W_out[dr].astype(np.float32) @ Wg[dr * Dm:(dr + 1) * Dm, :]
           for dr in range(2)]
    wfv = [W_out[dr].astype(np.float32) @ Wv[dr * Dm:(dr + 1) * Dm, :]
           for dr in range(2)]
    for core in range(NCORES):
        b, dr, s = core >> 2, (core >> 1) & 1, core & 1
        sl = slice(s * DiS, (s + 1) * DiS)
        W_eff = ln_w[dr][:, None] * W_in[dr]                         # [Dm, 4096]
        b_eff = ln_b[dr] @ W_in[dr] + b_in[dr]                       # [4096]
        # xp channel permutation: this core's Di-shard channels come first
        perm = np.concatenate([np.arange(s * DiS, (s + 1) * DiS),
                               np.arange((1 - s) * DiS, (2 - s) * DiS)])
        cols = np.concatenate([perm, Di + s * DiS + np.arange(DiS)])
        Wc = W_eff[:, cols]                                          # [Dm, 3072]
        in_maps.append({
            "xT": (xf if dr == 0 else xr)[b],
            "w_in": Wc.astype(BF),
            "w_in_c": (-Wc.sum(0, keepdims=True)).astype(BF),
            "b_in": b_eff[cols][:, None].astype(np.float32),
            "conv_w": conv_w[dr][perm].astype(np.float32),
            "conv_b": conv_b[dr][perm][:, None].astype(np.float32),
            "w_xp": W_xproj[dr][perm].astype(BF),
            "w_dt": W_dt[dr][:, sl].astype(BF),
            "b_dt": b_dt[dr][sl][:, None].astype(np.float32),
            "a_mat": (-np.exp(A_log[dr][sl])).astype(np.float32),
            "d_vec": D[dr][sl][:, None].astype(np.float32),
            "w_out": W_out[dr][sl, :].astype(BF),
            "wg_h": wg_bf[dr],
            "wv_h": wv_bf[dr],
            "wx_h": wg_bf[dr] if s == 0 else wv_bf[dr],
            "w_fg": wfg[dr][sl, :].astype(BF),
            "w_fv": wfv[dr][sl, :].astype(BF),
            "ident": ident_np,
        })
    return in_maps, xf


def postprocess(res1, x, b_out, Wg, bg, Wv, bv):
    """Host combine: sums of partials, sigmoid gate, convex blend."""
    idx = lambda b, dr, s: (b << 2) | (dr << 1) | s
    out = np.empty((B, L, Dm), np.float32)
    f32 = np.float32
    bias_g = (b_out[0] @ Wg[:Dm] + b_out[1] @ Wg[Dm:] + bg).astype(f32)
    bias_v = (b_out[0] @ Wv[:Dm] + b_out[1] @ Wv[Dm:] + bv).astype(f32)
    def psum3(core, key):
        r = res1[core][key]
        return (r[0].astype(f32) + r[1].astype(f32) + r[2].astype(f32))

    for b in range(B):
        pf = psum3(idx(b, 0, 0), "p_out") + psum3(idx(b, 0, 1), "p_out")
        pb = (psum3(idx(b, 1, 0), "p_out")
              + psum3(idx(b, 1, 1), "p_out"))[:, ::-1]
        xb = x[b].T.astype(f32)                               # [Dm, L]
        fwd = xb + b_out[0][:, None] + pf
        bwd = xb + b_out[1][:, None] + pb
        s_sum = fwd + bwd
        pre_g = (psum3(idx(b, 0, 0), "pre_a") + psum3(idx(b, 0, 1), "pre_a")
                 + psum3(idx(b, 1, 0), "pre_a")[:, ::-1]
                 + psum3(idx(b, 1, 1), "pre_a")[:, ::-1]
                 + res1[idx(b, 0, 0)]["pre_x"]
                 + res1[idx(b, 1, 0)]["pre_x"][:, ::-1])
        pre_v = (psum3(idx(b, 0, 0), "pre_b") + psum3(idx(b, 0, 1), "pre_b")
                 + psum3(idx(b, 1, 0), "pre_b")[:, ::-1]
                 + psum3(idx(b, 1, 1), "pre_b")[:, ::-1]
                 + res1[idx(b, 0, 1)]["pre_x"]
                 + res1[idx(b, 1, 1)]["pre_x"][:, ::-1])
        g = 1.0 / (1.0 + np.exp(-(pre_g + bias_g[:, None])))
        v = pre_v + bias_v[:, None]
        out[b] = (0.5 * (g * (v - s_sum) + s_sum)).T
    return out


def kernel(x, ln_w, ln_b, W_in, b_in, conv_w, conv_b, W_xproj, W_dt, b_dt,
           A_log, D, W_out, b_out, Wg, bg, Wv, bv):
    x = np.asarray(x, np.float32)
    args = [np.asarray(a, np.float32) for a in
            (ln_w, ln_b, W_in, b_in, conv_w, conv_b, W_xproj, W_dt, b_dt,
             A_log, D, W_out, b_out)]
    Wg, bg, Wv, bv = (np.asarray(a, np.float32) for a in (Wg, bg, Wv, bv))

    in1, xf = prep_launch1_inmaps(x, *args, Wg, Wv)
    nc1 = _get_nc(1)
    res1 = run_bass_kernel_spmd(nc1, in1, core_ids=list(range(NCORES))).results
    return postprocess(res1, x, args[-1], Wg, bg, Wv, bv)
